# revision 34
# baseline (speedup 1.0000x reference)
"""Trainium2 Bass kernel: pre-LN transformer block (B=4, T=2048, E=1024, H=16, FFN=100).

Sharding (8 NeuronCores): core 2b+g handles batch b, head-group g (8 of 16
heads, i.e. a 512-wide slice of the QKV output dim / proj input dim).  Both
cores of a pair compute attention for all 2048 tokens of their batch; after
each chunk's attention the pair exchanges normalized attention outputs
(attT) for the tokens the *other* core owns via a zero-masked pair
ReduceScatter (each core's contribution to its own slot is multiplied by a
per-core 0/1 input mask, so the RS-add delivers exactly the peer's attT) —
256KB on the wire per chunk instead of the 1MB proj-partial RS, and it fires
*before* proj, so proj + residual + LN2 + FFN for the core's own 256-token
shard run with no collective behind them.  All rank-dependent choices (which
token half is "mine", the proj weight row order, residual rows) live in
per-core input data, keeping the single SPMD program rank-symmetric.

Schedule: a burst of throwaway warm-up matmuls at t~0 flips the PE HAM clock
gate to 8/8 before real work lands; chunk 0's x subtiles and wq stream first
so LN1+QKV start ~15us in.  All four chunks' LN1+QKV run first (dense PE
work), then attention per chunk; chunk c's tail (peer attT load + proj + FFN)
is emitted inside chunk c+1's attention stream so the PE never idles and
every RS overlaps later attention; the last chunk's RS is split in two so
its tail starts earlier.  x is loaded in bf16 (it only feeds LN1); the f32
residual rows arrive separately as x_own (with b_proj folded in host-side).

Attention: scores are computed transposed, S^T[t_k, t_q] = k^T.T @ q^T, with
q^T/k^T in [head_dim, token] layout (from PE-transposed bf16 LN output).  The
two heads of a d-tile pair occupy partitions 0-63 / 64-127 and run as
concurrent row-group matmuls into one 2-bank PSUM tile, so a single ScalarE
exp (1/sqrt(E) scale folded in) covers both.  Diagonal t_k tiles compute only
the causally live columns (shorter score/AV matmuls + sliced exp) and one
TS-wide mask multiply; AV matmuls trail the exp stream by two tiles.  The
softmax denominator comes from a ones column appended to V; its reciprocal is
exp(-ln(den)) on ScalarE after a tiny SBUF->SBUF DMA gathers both heads'
denominator rows onto two partitions (a [1, N] activation would serialize on
one lane), and is broadcast across the head's 64 partitions with a K=1
ones-matmul into PSUM.  LayerNorm rsqrt is exp(-0.5*ln(var+eps)), keeping
the whole kernel on a single ScalarE table set (natural_log_exp_and_others).
"""

from contextlib import ExitStack

import numpy as np
import ml_dtypes

import concourse.bass as bass
import concourse.mybir as mybir
import concourse.tile as tile
from concourse.bass_utils import run_bass_kernel_spmd
from concourse.vector_clock import ScopedClock


class SplitDrainTC(tile.TileContext):
    """Works around a walrus codegen limit: an SP CTRL instruction may carry
    only one sync wait, so the kernel-tail drain's waits are split onto
    preceding single-wait nops."""

    def _drain_and_barrier(self, tick_clock, wait_clock):
        probe = self.nc.sync.nop(nofuse=True)
        wait_clock.add_sem_waits(
            probe.ins, ScopedClock({None: tick_clock.global_clock})
        )
        si = probe.ins.sync_info
        waits = list(si.on_wait) if si is not None else []
        if len(waits) > 1:
            si.on_wait = [waits[0]]
            for w in waits[1:]:
                n2 = self.nc.sync.nop(nofuse=True)
                n2.ins.sync_info = mybir.SyncInfo(on_wait=[w], on_update=[])
        self.nc.sync.drain()
        self.nc.all_engine_barrier()
        popped = self.nc._tile_sem_poison_stack.pop()
        assert popped is self._sem_poison
        self.nc.clear_and_free_semaphores(list(self.sems.allocated().values()))
        self.nc.all_engine_barrier()

B, T, E, H, HS, FFN = 4, 2048, 1024, 16, 64, 100
EPS = 1e-5
NCORE = 8
TC = 512            # token chunk
NTC = T // TC       # 4
TS = 128            # token subtile
NSUB = TC // TS     # 4
ET = 128            # embed tile
NET = E // ET       # 8
DSL = E // 2        # per-core qkv output slice (8 heads * 64)
NDT = DSL // 128    # 4 d-tiles (2 heads each)
HPC = H // 2        # 8 heads per core
HT = TC // 2        # 256: tokens owned per core per chunk
SCALE = float(E) ** -0.5
PAIRS = [[0, 1], [2, 3], [4, 5], [6, 7]]

MM_MODE = "bf16"    # "bf16" | "f32r" | "f32"
AF = mybir.ActivationFunctionType


def _mdt(mode):
    return mybir.dt.bfloat16 if mode == "bf16" else mybir.dt.float32


def _np_mdt(mode):
    return ml_dtypes.bfloat16 if mode == "bf16" else np.float32


def build(mode=MM_MODE):
    f32 = mybir.dt.float32
    mdt = _mdt(mode)

    def mc(ap):
        """Cast an AP for use as a matmul operand."""
        if mode == "f32r":
            return ap.bitcast(mybir.dt.float32r)
        return ap

    nc = bass.Bass(num_devices=NCORE)

    io = {}

    def param(name, shape, dtype):
        io[name] = nc.declare_dram_parameter(name, shape, dtype, isOutput=False)

    # x pre-shuffled host-side so one chunk is one [128, 4*E] DMA with 8KB
    # contiguous per partition: x[c, p, s*E:(s+1)*E] = x_orig[c*512+s*128+p]
    param("x", [NTC, TS, NSUB * E], mdt)     # bf16: only feeds LN1
    param("x_own", [NTC, TS, 2 * E], f32)    # own residual rows, + b_proj
    param("consts", [128, 20], f32)          # ln1g | ln1b | selmask packed
    param("wq", [E, DSL], mdt)
    param("wk", [E, DSL], mdt)
    param("wv", [E, DSL], mdt)
    param("wp", [E, E], mdt)                 # rows reordered: [own 512; peer 512]
    param("w1", [E, FFN], mdt)
    param("w2e", [FFN + 1, E], mdt)    # w2 with b2 as the extra last row
    param("b1", [FFN, 1], f32)
    param("ln1g", [E, 1], f32)
    param("ln1b", [E, 1], f32)
    param("ln2g", [E, 1], f32)
    param("ln2b", [E, 1], f32)
    param("masks", [TS, NSUB, TC], mdt)
    param("ident", [TS, TS], mdt)
    param("bsel", [2, TS], mdt)
    io["out"] = nc.declare_dram_parameter(
        "out", [NTC, HT, E], f32, isOutput=True
    )

    with SplitDrainTC(nc) as tc:
        with ExitStack() as ctx:
            _build_tile(ctx, tc, nc, mode, mdt, f32, mc, io)
    _split_waits(nc)
    return nc


def _split_waits(nc, maxw=1):
    """walrus codegen accepts a limited number of sync waits per instruction;
    move the excess onto same-engine NoOps inserted just before."""
    import bass_rust
    n = 0
    for f in nc.m.functions:
        for b in f.blocks:
            new = []
            for inst in b.instructions:
                si = inst.sync_info
                # fixed-length ISA instructions can't carry waits at all
                cap = 0 if isinstance(inst, bass_rust.InstISA) else maxw
                if si is not None and len(si.on_wait) > cap:
                    waits = list(si.on_wait)
                    keep = waits[-cap:] if cap else []
                    excess = waits[:-cap] if cap else waits
                    for w in excess:
                        nop = mybir.InstNoOp(
                            name=f"{inst.name}-wsplit{n}", engine=inst.engine
                        )
                        nop.bass_nofuse = True
                        n += 1
                        nop.sync_info = mybir.SyncInfo(
                            on_wait=[w], on_update=[]
                        )
                        new.append(nop)
                    si.on_wait = keep
                new.append(inst)
            if n:
                b.instructions = new


def _build_tile(ctx, tc, nc, mode, mdt, f32, mc, io):
    x, out = io["x"], io["out"]

    def pool(name, bufs, space="SBUF"):
        return ctx.enter_context(tc.tile_pool(name=name, bufs=bufs, space=space))

    # ---- internal DRAM: per-chunk attT-exchange RS buffers.  agi[j] holds
    # this core's attT columns for the tokens rank j owns (own-dest slot
    # zero-masked); the pair RS-add delivers the peer's attT for my tokens.
    dram = pool("dram", 1, space="DRAM")
    agi_c = [dram.tile([2, NDT, TS, HT], mdt, name=f"agi{c}") for c in range(3)]
    ago_c = [dram.tile([NDT, TS, HT], mdt, name=f"ago{c}") for c in range(3)]
    # last chunk: two half-exchanges (d-tiles 0-1 / 2-3) so its tail starts
    # as soon as the first half's attention pairs finish
    agi_3 = [dram.tile([2, 2, TS, HT], mdt, name=f"agi3{h}") for h in range(2)]
    ago_3 = [dram.tile([2, TS, HT], mdt, name=f"ago3{h}") for h in range(2)]

    # ---- persistent SBUF: weights & constants.  Emission order sets the
    # tile scheduler's priority: chunk 0's x (one packed 1MB DMA) + the
    # consts pack + wq go first so LN1+QKV start early. ----
    wpool = pool("weights", 1)
    xt_pool = pool("xt", 2)        # [128, 4*E] bf16: one chunk of x rows

    def prefetch_x(c):
        x_t = xt_pool.tile([TS, NSUB * E], mdt, name="x_t")
        nc.sync.dma_start(out=x_t, in_=x[c])
        return x_t

    x0 = prefetch_x(0)
    warm_src = wpool.tile([128, 128], mdt, name="warm_src")
    nc.vector.memset(warm_src, 0.25)
    warm_mv = wpool.tile([128, TC], mdt, name="warm_mv")
    nc.vector.memset(warm_mv, 0.25)
    consts = wpool.tile([128, 20], f32, name="consts")
    nc.scalar.dma_start(out=consts, in_=io["consts"][:])
    ln_sb = {"ln1g": consts[:, 0:8], "ln1b": consts[:, 8:16]}
    # cols 0-1: RS staging sel (0 for my own dest slot); cols 2-3: own-token
    # half select (1 for my half)
    sel_sb = consts[:, 16:20]
    id_sb = wpool.tile([TS, TS], mdt, name="id_sb")
    nc.scalar.dma_start(out=id_sb, in_=io["ident"][:])
    # block "selector" for the denominator broadcast: one K=2 matmul maps
    # dr2 [2, TC] onto [128, TC] with head h's reciprocal on partitions
    # h*64..h*64+63 (host-provided constant)
    bsel = wpool.tile([2, 128], mdt, name="bsel")
    nc.scalar.dma_start(out=bsel, in_=io["bsel"][:])
    wq_sb = wpool.tile([ET, NET, DSL], mdt, name="wq_sb")
    wk_sb = wpool.tile([ET, NET, DSL], mdt, name="wk_sb")
    wv_sb = wpool.tile([ET, NET, DSL], mdt, name="wv_sb")
    nc.gpsimd.dma_start(out=wq_sb, in_=io["wq"].rearrange("(k p) d -> p k d", p=ET))
    nc.gpsimd.dma_start(out=wk_sb, in_=io["wk"].rearrange("(k p) d -> p k d", p=ET))
    nc.gpsimd.dma_start(out=wv_sb, in_=io["wv"].rearrange("(k p) d -> p k d", p=ET))
    eps_sb = wpool.tile([128, 1], f32, name="eps_sb")
    nc.vector.memset(eps_sb, EPS)
    mask_sb = wpool.tile([TS, NSUB, TC], mdt, name="mask_sb")
    wp_sb = wpool.tile([128, 2 * NDT, E], mdt, name="wp_sb")
    w1_sb = wpool.tile([ET, NET, FFN], mdt, name="w1_sb")
    w2_sb = wpool.tile([FFN + 1, E], mdt, name="w2_sb")
    b1_sb = wpool.tile([FFN, 1], f32, name="b1_sb")

    def load_late_weights():
        nc.gpsimd.dma_start(out=mask_sb, in_=io["masks"][:])
        nc.gpsimd.dma_start(
            out=wp_sb, in_=io["wp"].rearrange("(k p) d -> p k d", p=128)
        )
        nc.gpsimd.dma_start(
            out=w1_sb, in_=io["w1"].rearrange("(k p) d -> p k d", p=ET)
        )
        nc.gpsimd.dma_start(out=w2_sb, in_=io["w2e"][:])
        nc.gpsimd.dma_start(out=b1_sb, in_=io["b1"][:])
        for nm in ("ln2g", "ln2b"):
            t = wpool.tile([ET, NET], f32, name=nm + "_sb")
            nc.gpsimd.dma_start(
                out=t, in_=io[nm].rearrange("(k p) o -> p (k o)", p=ET)
            )
            ln_sb[nm] = t[:, :]

    # ---- persistent SBUF: per-chunk K^T, V(+ones), Q^T ----
    kv = pool("kv", 1)
    kT_c = [kv.tile([128, NDT, TC], mdt, name=f"kT{c}") for c in range(NTC)]
    vt_c = [kv.tile([128, NSUB, HPC, HS + 1], mdt, name=f"vt{c}")
            for c in range(NTC)]
    qT_c = [kv.tile([128, NDT, TC], mdt, name=f"qT{c}") for c in range(NTC)]

    # ---- working pools ----
    h_pool = pool("h", 6)          # [128, E] bf16: LN output rows
    mv_pool = pool("mv", 3)
    hT_pool = pool("hT", 2)        # [128, NET, TC] bf16
    pt_pool = pool("pt", 5)        # [128, 2, TC] bf16 softmax tiles
    avs_pool = pool("avs", 3)      # [HS+1, 2, TC] bf16
    dr_pool = pool("dr", 2)        # [2, TC] denominators / reciprocals
    attT_pool = pool("attT", 5)    # [128, TC] bf16
    own_pool = pool("own", 6)      # [128, HT] bf16: own-token attT columns
    stg_pool = pool("stg", 3)      # [128, 2, HT] bf16: RS staging
    peer_pool = pool("peer", 2)    # [128, NDT, HT] bf16: peer attT
    x2_pool = pool("x2", 1)        # [128, 2, E] f32: phase-3 residual rows
    f1_pool = pool("f1", 2)
    out_pool = pool("outp", 2)
    ps_mm = pool("ps_mm", 2, space="PSUM")   # [128, 512] (1 bank each)
    ps_sc = pool("ps_sc", 2, space="PSUM")   # [128, 2, 512] (2 banks each)
    ps_av = pool("ps_av", 2, space="PSUM")   # [HS+1, 512] (1 bank each)

    # ---- HAM warm-up: ~9us of back-to-back throwaway matmuls (one PSUM
    # accumulation group -> no inter-matmul semaphores) so the PE clock gate
    # is at 8/8 and stays there until the first real QKV work arrives ----
    def warmup(n=22):
        psw = ps_mm.tile([128, TC], f32, name="psw", tag="mm")
        for i in range(n):
            nc.tensor.matmul(psw, mc(warm_src), mc(warm_mv),
                             start=(i == 0), stop=(i == n - 1))

    def layer_norm(x_t, out_t):
        """out_t (bf16) = (x - mean) * rsqrt(var + eps).
        rsqrt is exp(-0.5*ln(var+eps)) to stay in one ScalarE table set."""
        stats = mv_pool.tile([128, 2, nc.vector.BN_STATS_DIM], f32, name="stats")
        xg = x_t.rearrange("p (s q) -> p s q", s=2)
        for s in range(2):
            nc.vector.bn_stats(out=stats[:, s, :], in_=xg[:, s, :])
        mv = mv_pool.tile([128, 2], f32, name="mv")
        nc.vector.bn_aggr(out=mv, in_=stats)
        rstd = mv_pool.tile([128, 1], f32, name="rstd")
        nc.scalar.activation(
            out=rstd, in_=mv[:, 1:2], func=AF.Ln, bias=eps_sb, scale=1.0
        )
        nc.scalar.activation(out=rstd, in_=rstd, func=AF.Exp, scale=-0.5)
        nc.vector.tensor_scalar(
            out=out_t, in0=x_t, scalar1=mv[:, 0:1], scalar2=rstd,
            op0=mybir.AluOpType.subtract, op1=mybir.AluOpType.mult,
        )
        return out_t

    def transpose_cast(h_ts, g_sb, b_sb, hT, width, keep_warm=False):
        """PE-transpose len(h_ts) subtiles of h [128, E] into hT[:, k, :]
        (bf16), batching all of them into one PSUM tile per e-tile so the
        layernorm scale/bias fold costs one DVE op per [128, width].
        keep_warm: sprinkle throwaway matmuls between e-tile groups so the
        PE HAM clock stays at 8/8 while the LN1 stream trickles in."""
        nsub = len(h_ts)
        for k in range(NET):
            tp = ps_mm.tile([TS, nsub * TS], mdt, name="tp", tag="mm")
            for s in range(nsub):
                nc.tensor.transpose(
                    tp[:, s * TS:(s + 1) * TS],
                    h_ts[s][:, k * ET:(k + 1) * ET], id_sb,
                )
            if keep_warm:
                warmup(4)
            nc.vector.tensor_scalar(
                out=hT[:, k, 0:width], in0=tp,
                scalar1=g_sb[:, k:k + 1], scalar2=b_sb[:, k:k + 1],
                op0=mybir.AluOpType.mult, op1=mybir.AluOpType.add,
            )

    # =====================================================================
    # Phase 1: LN1 + transpose + QKV per chunk
    # =====================================================================
    def ln1_from(x4):
        h_ts = []
        for s in range(NSUB):
            h_t = h_pool.tile([128, E], mdt, name="h_t")
            h_ts.append(layer_norm(x4[:, s * E:(s + 1) * E], h_t))
        return h_ts

    def qkv_hT(h_ts, keep_warm=False):
        hT = hT_pool.tile([ET, NET, TC], mdt, name="hT")
        transpose_cast(h_ts, ln_sb["ln1g"], ln_sb["ln1b"], hT, TC,
                       keep_warm=keep_warm)
        return hT

    def qkv_mms(c, hT):
        for dd in range(NDT):
            for w_sb, dst in ((wq_sb, qT_c[c]), (wk_sb, kT_c[c])):
                ps = ps_mm.tile([128, TC], f32, name="ps_qk", tag="mm")
                for k in range(NET):
                    nc.tensor.matmul(
                        ps, mc(w_sb[:, k, dd * 128:(dd + 1) * 128]),
                        mc(hT[:, k, :]),
                        start=(k == 0), stop=(k == NET - 1),
                    )
                nc.vector.tensor_copy(dst[:, dd, :], ps)
        for s in range(NSUB):
            ps = ps_mm.tile([128, DSL], f32, name="ps_v", tag="mm")
            for k in range(NET):
                nc.tensor.matmul(
                    ps, mc(hT[:, k, s * TS:(s + 1) * TS]), mc(wv_sb[:, k, :]),
                    start=(k == 0), stop=(k == NET - 1),
                )
            nc.vector.tensor_copy(
                vt_c[c][:, s, :, 0:HS],
                ps.rearrange("p (h d) -> p h d", h=HPC),
            )
            nc.vector.memset(vt_c[c][:, s, :, HS:HS + 1], 1.0)

    # =====================================================================
    # Phase 2: attention + attT normalize + pair exchange
    # =====================================================================
    def finish_pair(av_sb, dr2, attT):
        """Broadcast the reciprocal denominators across each head's 64
        partitions with a K=2 selector-matmul and normalize into attT."""
        rb = ps_mm.tile([128, TC], f32, name="rb", tag="mm")
        nc.tensor.matmul(rb, mc(bsel), mc(dr2), start=True, stop=True)
        for hh in range(2):
            nc.vector.tensor_mul(
                attT[hh * HS:(hh + 1) * HS, :], av_sb[0:HS, hh, :],
                rb[hh * HS:(hh + 1) * HS, :],
            )

    def stage_pair(c, pr, attT):
        """Write attT's two token-half column blocks to the RS staging DRAM
        (own-dest slot zeroed via selmask) and produce the own-token columns
        for the local proj."""
        stg = stg_pool.tile([128, 2, HT], mdt, name="stg")
        for j in range(2):
            nc.vector.tensor_scalar_mul(
                stg[:, j, :], attT[:, j * HT:(j + 1) * HT],
                sel_sb[:, j:j + 1],
            )
        if c == NTC - 1:
            dst = agi_3[pr // 2][:, pr % 2, :, :]
        else:
            dst = agi_c[c][:, pr, :, :]
        nc.sync.dma_start(out=dst.rearrange("j p f -> p j f"), in_=stg)
        # own-token columns: attT[:, g*256:(g+1)*256] selected via input data
        ow = own_pool.tile([128, HT], mdt, name="ow")
        tmp = stg_pool.tile([128, HT], mdt, name="owt", tag="owt")
        nc.vector.tensor_scalar_mul(tmp, attT[:, 0:HT], sel_sb[:, 2:3])
        nc.vector.tensor_scalar_mul(ow, attT[:, HT:2 * HT], sel_sb[:, 3:4])
        nc.vector.tensor_add(ow, ow, tmp)
        return ow

    def rs_chunk(c, half=None):
        if c == NTC - 1:
            nc.gpsimd.collective_compute(
                "ReduceScatter", mybir.AluOpType.add, replica_groups=PAIRS,
                ins=[agi_3[half][:]], outs=[ago_3[half][:]],
            )
        else:
            nc.gpsimd.collective_compute(
                "ReduceScatter", mybir.AluOpType.add, replica_groups=PAIRS,
                ins=[agi_c[c][:]], outs=[ago_c[c][:]],
            )

    def attention_chunk(c, fillers=None):
        nkt = (c + 1) * NSUB
        owns = []
        pending = None

        def finish_stage(pending, pr_done):
            finish_pair(*pending)
            owns.append(stage_pair(c, pr_done, pending[2]))
            if c == NTC - 1 and pr_done == 1:
                rs_chunk(c, half=0)
            if pr_done == NDT - 1:
                rs_chunk(c, half=1 if c == NTC - 1 else None)

        for pr in range(NDT):  # head pair = d-tile
            fn = (fillers or {}).pop(pr, None)
            if fn is not None:
                fn()
            av_ps = [ps_av.tile([HS + 1, TC], f32, name="avp") for _ in range(2)]
            def av_mms(pi, ppt, p0, last):
                # columns below p0 get no contribution from this t_k tile
                # (fully above the diagonal); PSUM accumulation is
                # per-element so the shorter matmul leaves them untouched
                for hh in range(2):
                    nc.tensor.matmul(
                        av_ps[hh][:, p0:TC],
                        mc(vt_c[pi // NSUB][:, pi % NSUB, pr * 2 + hh, :]),
                        mc(ppt[:, hh, p0:TC]),
                        start=(pi == 0), stop=last,
                    )

            avq = []  # stagger AV matmuls 2 units behind exp+mask
            for i in range(nkt):
                m = i - c * NSUB
                # p0: first t_q column this t_k tile can attend to
                p0 = m * TS if m > 0 else 0
                sc2 = ps_sc.tile([TS, 2, TC], f32, name="sc2")
                for hh in range(2):
                    h0 = hh * HS
                    nc.tensor.matmul(
                        sc2[:, hh, p0:TC],
                        mc(kT_c[i // NSUB][h0:h0 + HS, pr,
                                           (i % NSUB) * TS:(i % NSUB + 1) * TS]),
                        mc(qT_c[c][h0:h0 + HS, pr, p0:TC]),
                        start=True, stop=True,
                    )
                pt2 = pt_pool.tile([TS, 2, TC], mdt, name="pt2")
                nc.scalar.activation(
                    out=pt2[:, :, p0:TC], in_=sc2[:, :, p0:TC],
                    func=AF.Exp, scale=SCALE,
                )
                if m >= 0:
                    # diagonal TS block: zero t_k > t_q within it
                    for hh in range(2):
                        nc.vector.tensor_mul(
                            pt2[:, hh, p0:p0 + TS], pt2[:, hh, p0:p0 + TS],
                            mask_sb[:, m, p0:p0 + TS],
                        )
                avq.append((i, pt2, p0))
                if len(avq) > 2:
                    av_mms(*avq.pop(0), last=False)
                if i == 3 and pending is not None:
                    finish_stage(pending, pr - 1)
                    pending = None
            while avq:
                av_mms(*avq.pop(0), last=(len(avq) == 0))
            av_sb = avs_pool.tile([HS + 1, 2, TC], mdt, name="av_sb")
            for hh in range(2):
                nc.vector.tensor_copy(av_sb[:, hh, :], av_ps[hh])
            # 1/den on ScalarE as exp(-ln(den)); the two heads' denominator
            # rows are DMA-gathered onto two partitions first so the
            # activation runs on 2 lanes instead of 1
            den2 = dr_pool.tile([2, TC], mdt, name="den2", tag="den")
            for hh in range(2):
                nc.sync.dma_start(
                    out=den2[hh:hh + 1, :], in_=av_sb[HS:HS + 1, hh, :]
                )
            lden = dr_pool.tile([2, TC], f32, name="lden", tag="lden")
            nc.scalar.activation(out=lden, in_=den2, func=AF.Ln)
            dr2 = dr_pool.tile([2, TC], mdt, name="dr2", tag="dr")
            nc.scalar.activation(out=dr2, in_=lden, func=AF.Exp, scale=-1.0)
            attT = attT_pool.tile([128, TC], mdt, name="attT")
            if pending is not None:
                finish_stage(pending, pr - 1)
            pending = (av_sb, dr2, attT)
        finish_stage(pending, NDT - 1)
        return owns

    # =====================================================================
    # Phase 3: peer attT + proj + residual + LN2 + FFN on own 256 tokens
    # =====================================================================
    def tail_chunk(c, owns, peer_waits=None):
        """proj for this core's 256-token shard (contraction over own 4 +
        peer 4 d-tiles), then residual + LN2 + FFN + output."""
        # peer/x_own/out go on the gpsimd DMA queue: it carries the
        # collectives, so the peer load orders naturally behind its RS and
        # none of these (which can wait multi-us on data) block the sync
        # queue that carries the attention-critical den/stage transfers
        peer = peer_pool.tile([128, NDT, HT], mdt, name="peer")
        if c == NTC - 1:
            for h in range(2):
                nc.gpsimd.dma_start(
                    out=peer[:, 2 * h:2 * h + 2, :],
                    in_=ago_3[h].rearrange("d p f -> p d f"),
                )
        else:
            nc.gpsimd.dma_start(
                out=peer, in_=ago_c[c].rearrange("d p f -> p d f")
            )
        x2 = x2_pool.tile([TS, 2, E], f32, name="x2_t")
        nc.scalar.dma_start(out=x2, in_=io["x_own"][c])
        x2_ts = []
        h2_ts = []
        for s in range(2):
            x2_t = x2[:, s, :]
            for n in range(2):
                ps = ps_mm.tile([128, TC], f32, name="ps_pr", tag="mm")
                for dd in range(NDT):
                    nc.tensor.matmul(
                        ps, mc(owns[dd][:, s * TS:(s + 1) * TS]),
                        mc(wp_sb[:, dd, n * TC:(n + 1) * TC]),
                        start=(dd == 0), stop=False,
                    )
                for dd in range(NDT):
                    nc.tensor.matmul(
                        ps, mc(peer[:, dd, s * TS:(s + 1) * TS]),
                        mc(wp_sb[:, NDT + dd, n * TC:(n + 1) * TC]),
                        start=False, stop=(dd == NDT - 1),
                    )
                nc.vector.tensor_add(
                    x2_t[:, n * TC:(n + 1) * TC], x2_t[:, n * TC:(n + 1) * TC],
                    ps,
                )
            x2_ts.append(x2_t)
            h2_t = h_pool.tile([128, E], mdt, name="h2_t", tag="h_t")
            layer_norm(x2_t, h2_t)
            h2_ts.append(h2_t)
        h2T = hT_pool.tile([ET, NET, HT], mdt, name="h2T")
        transpose_cast(h2_ts, ln_sb["ln2g"], ln_sb["ln2b"], h2T, HT)
        f1 = f1_pool.tile([FFN + 1, HT], mdt, name="f1")
        nc.vector.memset(f1, 1.0)  # row FFN stays 1.0 (b2 matmul row)
        ps_f = ps_mm.tile([FFN, HT], f32, name="ps_f", tag="mm")
        for k in range(NET):
            nc.tensor.matmul(
                ps_f, mc(w1_sb[:, k, :]), mc(h2T[:, k, :]),
                start=(k == 0), stop=(k == NET - 1),
            )
        nc.scalar.activation(
            out=f1[0:FFN, :], in_=ps_f, func=AF.Relu, bias=b1_sb, scale=1.0
        )
        for s in range(2):
            o_t = out_pool.tile([128, E], f32, name="o_t")
            for n in range(2):
                ps = ps_mm.tile([128, TC], f32, name="ps_o", tag="mm")
                nc.tensor.matmul(
                    ps, mc(f1[:, s * TS:(s + 1) * TS]),
                    mc(w2_sb[:, n * TC:(n + 1) * TC]),
                    start=True, stop=True,
                )
                nc.vector.tensor_add(
                    o_t[:, n * TC:(n + 1) * TC], ps,
                    x2_ts[s][:, n * TC:(n + 1) * TC],
                )
            nc.scalar.dma_start(out=out[c, s * TS:(s + 1) * TS, :], in_=o_t)

    # ---- schedule ----
    warmup(12)
    load_late_weights()
    qkv_mms(0, qkv_hT(ln1_from(x0), keep_warm=True))
    for c in range(1, NTC):
        qkv_mms(c, qkv_hT(ln1_from(prefetch_x(c))))
    pending_tail = None
    for c in range(NTC):
        fillers = {}
        if pending_tail is not None:
            fillers[2] = pending_tail
        owns = attention_chunk(c, fillers=fillers)
        pending_tail = (lambda cc, oo: lambda: tail_chunk(cc, oo))(c, owns)
    pending_tail()


# =========================================================================
# Host side
# =========================================================================
def _make_masks(np_mdt):
    # masks[p, d, f] = 1 iff t_k <= t_q for the diagonal block at offset d,
    # i.e. f >= 128*d + p  (t_k = 128*i + p, t_q = 512*c + f, i = 4*c + d)
    m = np.zeros((TS, NSUB, TC), dtype=np.float32)
    for d in range(NSUB):
        for p in range(TS):
            m[p, d, d * TS + p:] = 1.0
    return m.astype(np_mdt)


_NC_CACHE = {}
RUN_KWARGS = {}      # test harness may set {"trace": True} for profiling
LAST_RESULT = None   # BassKernelResults of the most recent run


def kernel(x, wq, wk, wv, w_proj, b_proj, w1, b1, w2, b2, ln1_g, ln1_b, ln2_g,
           ln2_b):
    mode = MM_MODE
    np_mdt = _np_mdt(mode)
    if mode not in _NC_CACHE:
        _NC_CACHE[mode] = build(mode)
    nc = _NC_CACHE[mode]

    x = np.asarray(x, np.float32)
    bp = np.asarray(b_proj, np.float32)
    masks = _make_masks(np_mdt)
    identity = np.eye(TS, dtype=np.float32)
    w2e = np.concatenate([np.asarray(w2, np.float32),
                          np.asarray(b2, np.float32)[None, :]], axis=0)
    wp_full = np.asarray(w_proj, np.float32)
    bsel_np = np.zeros((2, TS), np.float32)
    bsel_np[0, 0:HS] = 1.0
    bsel_np[1, HS:TS] = 1.0

    def own_rows(c, g):
        return np.r_[c * TC + g * HT:c * TC + (g + 1) * HT]

    ln1g = np.asarray(ln1_g, np.float32)
    ln1b = np.asarray(ln1_b, np.float32)
    in_maps = []
    for core in range(NCORE):
        b, g = core // 2, core % 2
        sl = slice(g * DSL, (g + 1) * DSL)
        slp = slice((1 - g) * DSL, (2 - g) * DSL)
        # x_own packed [NTC, 128, 2*E]: x_own[c, p, s*E:] = own row s*128+p
        x_own = np.stack(
            [x[b, own_rows(c, g), :] for c in range(NTC)]
        ) + bp[None, None, :]
        x_own = x_own.reshape(NTC, 2, TS, E).transpose(0, 2, 1, 3) \
                     .reshape(NTC, TS, 2 * E)
        consts = np.zeros((128, 20), np.float32)
        consts[:, 0:8] = ln1g.reshape(8, 128).T
        consts[:, 8:16] = ln1b.reshape(8, 128).T
        consts[:, 16] = 0.0 if g == 0 else 1.0   # stage: zero own dest slot
        consts[:, 17] = 0.0 if g == 1 else 1.0
        consts[:, 18] = 1.0 if g == 0 else 0.0   # own-token half select
        consts[:, 19] = 1.0 if g == 1 else 0.0
        wp_core = np.concatenate([wp_full[sl, :], wp_full[slp, :]], axis=0)
        x_shuf = x[b].reshape(NTC, NSUB, TS, E).transpose(0, 2, 1, 3) \
                     .reshape(NTC, TS, NSUB * E)
        in_maps.append({
            "x": x_shuf.astype(np_mdt),
            "x_own": x_own,
            "consts": consts,
            "wq": np.asarray(wq, np.float32)[:, sl].astype(np_mdt),
            "wk": np.asarray(wk, np.float32)[:, sl].astype(np_mdt),
            "wv": np.asarray(wv, np.float32)[:, sl].astype(np_mdt),
            "wp": wp_core.astype(np_mdt),
            "w1": np.asarray(w1, np.float32).astype(np_mdt),
            "w2e": w2e.astype(np_mdt),
            "b1": np.asarray(b1, np.float32)[:, None],
            "ln1g": np.asarray(ln1_g, np.float32)[:, None],
            "ln1b": np.asarray(ln1_b, np.float32)[:, None],
            "ln2g": np.asarray(ln2_g, np.float32)[:, None],
            "ln2b": np.asarray(ln2_b, np.float32)[:, None],
            "masks": masks,
            "ident": identity.astype(np_mdt),
            "bsel": bsel_np.astype(np_mdt),
        })
    global LAST_RESULT
    res = run_bass_kernel_spmd(nc, in_maps, list(range(NCORE)), **RUN_KWARGS)
    LAST_RESULT = res
    outp = np.empty((B, T, E), np.float32)
    for core in range(NCORE):
        b, g = core // 2, core % 2
        o = res.results[core]["out"]
        for c in range(NTC):
            outp[b, own_rows(c, g), :] = o[c]
    return outp


# revision 46
# speedup vs baseline: 1.1191x; 1.1191x over previous
"""Trainium2 Bass kernel: pre-LN transformer block (B=4, T=2048, E=1024, H=16, FFN=100).

Sharding (8 NeuronCores): core 2b+g handles batch b, head-group g (8 of 16
heads, i.e. a 512-wide slice of the QKV output dim / proj input dim).  Both
cores of a pair compute attention for all 2048 tokens of their batch; after
each chunk's attention the pair exchanges normalized attention outputs
(attT) for the tokens the *other* core owns via a zero-masked pair
ReduceScatter (each core's contribution to its own slot is multiplied by a
per-core 0/1 input mask, so the RS-add delivers exactly the peer's attT) —
256KB on the wire per chunk instead of the 1MB proj-partial RS, and it fires
*before* proj, so proj + residual + LN2 + FFN for the core's own 256-token
shard run with no collective behind them.  All rank-dependent choices (which
token half is "mine", the proj weight row order, residual rows) live in
per-core input data, keeping the single SPMD program rank-symmetric.

Schedule: a burst of throwaway warm-up matmuls at t~0 flips the PE HAM clock
gate to 8/8 before real work lands; chunk 0's x subtiles and wq stream first
so LN1+QKV start ~15us in.  All four chunks' LN1+QKV run first (dense PE
work), then attention per chunk; chunk c's tail (peer attT load + proj + FFN)
is emitted inside chunk c+1's attention stream so the PE never idles and
every RS overlaps later attention; the last chunk's RS is split in two so
its tail starts earlier.  x is loaded in bf16 (it only feeds LN1); the f32
residual rows arrive separately as x_own (with b_proj folded in host-side).

Attention: scores are computed transposed, S^T[t_k, t_q] = k^T.T @ q^T, with
q^T/k^T in [head_dim, token] layout (from PE-transposed bf16 LN output).  The
two heads of a d-tile pair occupy partitions 0-63 / 64-127 and run as
concurrent row-group matmuls into one 2-bank PSUM tile, so a single ScalarE
exp (1/sqrt(E) scale folded in) covers both.  Diagonal t_k tiles compute only
the causally live columns (shorter score/AV matmuls + sliced exp) and one
TS-wide mask multiply; AV matmuls trail the exp stream by two tiles.  The
softmax denominator comes from a ones column appended to V; its reciprocal is
exp(-ln(den)) on ScalarE after a tiny SBUF->SBUF DMA gathers both heads'
denominator rows onto two partitions (a [1, N] activation would serialize on
one lane), and is broadcast across the head's 64 partitions with a K=1
ones-matmul into PSUM.  LayerNorm rsqrt is exp(-0.5*ln(var+eps)), keeping
the whole kernel on a single ScalarE table set (natural_log_exp_and_others).
"""

from contextlib import ExitStack

import numpy as np
import ml_dtypes

import concourse.bass as bass
import concourse.mybir as mybir
import concourse.tile as tile
from concourse.bass_utils import run_bass_kernel_spmd
from concourse.vector_clock import ScopedClock


class SplitDrainTC(tile.TileContext):
    """Works around a walrus codegen limit: an SP CTRL instruction may carry
    only one sync wait, so the kernel-tail drain's waits are split onto
    preceding single-wait nops."""

    def _drain_and_barrier(self, tick_clock, wait_clock):
        probe = self.nc.sync.nop(nofuse=True)
        wait_clock.add_sem_waits(
            probe.ins, ScopedClock({None: tick_clock.global_clock})
        )
        si = probe.ins.sync_info
        waits = list(si.on_wait) if si is not None else []
        if len(waits) > 1:
            si.on_wait = [waits[0]]
            for w in waits[1:]:
                n2 = self.nc.sync.nop(nofuse=True)
                n2.ins.sync_info = mybir.SyncInfo(on_wait=[w], on_update=[])
        self.nc.sync.drain()
        self.nc.all_engine_barrier()
        popped = self.nc._tile_sem_poison_stack.pop()
        assert popped is self._sem_poison
        self.nc.clear_and_free_semaphores(list(self.sems.allocated().values()))
        self.nc.all_engine_barrier()

B, T, E, H, HS, FFN = 4, 2048, 1024, 16, 64, 100
EPS = 1e-5
NCORE = 8
TC = 512            # token chunk
NTC = T // TC       # 4
TS = 128            # token subtile
NSUB = TC // TS     # 4
ET = 128            # embed tile
NET = E // ET       # 8
DSL = E // 2        # per-core qkv output slice (8 heads * 64)
NDT = DSL // 128    # 4 d-tiles (2 heads each)
HPC = H // 2        # 8 heads per core
HT = TC // 2        # 256: tokens owned per core per chunk
SCALE = float(E) ** -0.5
PAIRS = [[0, 1], [2, 3], [4, 5], [6, 7]]

MM_MODE = "bf16"    # "bf16" | "f32r" | "f32"
AF = mybir.ActivationFunctionType


def _mdt(mode):
    return mybir.dt.bfloat16 if mode == "bf16" else mybir.dt.float32


def _np_mdt(mode):
    return ml_dtypes.bfloat16 if mode == "bf16" else np.float32


def build(mode=MM_MODE):
    f32 = mybir.dt.float32
    mdt = _mdt(mode)

    def mc(ap):
        """Cast an AP for use as a matmul operand."""
        if mode == "f32r":
            return ap.bitcast(mybir.dt.float32r)
        return ap

    nc = bass.Bass(num_devices=NCORE)

    io = {}

    def param(name, shape, dtype):
        io[name] = nc.declare_dram_parameter(name, shape, dtype, isOutput=False)

    param("x", [T, E], mdt)                  # bf16: only feeds LN1
    param("x_own", [NTC, HT, E], f32)        # own residual rows, + b_proj
    # cols 0-1: RS staging sel (0 for my own dest slot); cols 2-3: own-token
    # half select (1 for my half)
    param("selmask", [128, 4], f32)
    param("wq", [E, DSL], mdt)
    param("wk", [E, DSL], mdt)
    param("wv", [E, DSL], mdt)
    param("wp", [E, E], mdt)                 # rows reordered: [own 512; peer 512]
    param("w1", [E, FFN], mdt)
    param("w2e", [FFN + 1, E], mdt)    # w2 with b2 as the extra last row
    param("b1", [FFN, 1], f32)
    param("ln1g", [E, 1], f32)
    param("ln1b", [E, 1], f32)
    param("ln2g", [E, 1], f32)
    param("ln2b", [E, 1], f32)
    param("masks", [TS, NSUB, TC], mdt)
    param("ident", [TS, TS], mdt)
    param("bsel", [2, TS], mdt)
    io["out"] = nc.declare_dram_parameter(
        "out", [NTC, HT, E], f32, isOutput=True
    )

    with SplitDrainTC(nc) as tc:
        with ExitStack() as ctx:
            _build_tile(ctx, tc, nc, mode, mdt, f32, mc, io)
    _split_waits(nc)
    return nc


def _split_waits(nc, maxw=1):
    """walrus codegen accepts a limited number of sync waits per instruction;
    move the excess onto same-engine NoOps inserted just before."""
    import bass_rust
    n = 0
    for f in nc.m.functions:
        for b in f.blocks:
            new = []
            for inst in b.instructions:
                si = inst.sync_info
                # fixed-length ISA instructions can't carry waits at all
                cap = 0 if isinstance(inst, bass_rust.InstISA) else maxw
                if si is not None and len(si.on_wait) > cap:
                    waits = list(si.on_wait)
                    keep = waits[-cap:] if cap else []
                    excess = waits[:-cap] if cap else waits
                    for w in excess:
                        nop = mybir.InstNoOp(
                            name=f"{inst.name}-wsplit{n}", engine=inst.engine
                        )
                        nop.bass_nofuse = True
                        n += 1
                        nop.sync_info = mybir.SyncInfo(
                            on_wait=[w], on_update=[]
                        )
                        new.append(nop)
                    si.on_wait = keep
                new.append(inst)
            if n:
                b.instructions = new


def _build_tile(ctx, tc, nc, mode, mdt, f32, mc, io):
    x, out = io["x"], io["out"]

    def pool(name, bufs, space="SBUF"):
        return ctx.enter_context(tc.tile_pool(name=name, bufs=bufs, space=space))

    # ---- internal DRAM: per-chunk attT-exchange RS buffers.  agi[j] holds
    # this core's attT columns for the tokens rank j owns (own-dest slot
    # zero-masked); the pair RS-add delivers the peer's attT for my tokens.
    dram = pool("dram", 1, space="DRAM")
    agi_c = [dram.tile([2, NDT, TS, HT], mdt, name=f"agi{c}") for c in range(3)]
    ago_c = [dram.tile([NDT, TS, HT], mdt, name=f"ago{c}") for c in range(3)]
    # last chunk: two half-exchanges (d-tiles 0-1 / 2-3) so its tail starts
    # as soon as the first half's attention pairs finish
    agi_3 = [dram.tile([2, 2, TS, HT], mdt, name=f"agi3{h}") for h in range(2)]
    ago_3 = [dram.tile([2, TS, HT], mdt, name=f"ago3{h}") for h in range(2)]

    # ---- persistent SBUF: weights & constants.  Emission order sets the
    # tile scheduler's priority: chunk 0's x subtiles + the small consts +
    # wq go first so LN1+QKV start early. ----
    wpool = pool("weights", 1)
    xt_pool = pool("xt", 4)        # [128, E] bf16: x rows for LN1

    def prefetch_x(c, spread=False):
        qs = [nc.sync, nc.scalar, nc.scalar, nc.sync] if spread \
            else [nc.sync] * NSUB
        x_ts = []
        for s in range(NSUB):
            r0 = c * TC + s * TS
            x_t = xt_pool.tile([128, E], mdt, name="x_t")
            qs[s].dma_start(out=x_t, in_=x[r0:r0 + TS, :])
            x_ts.append(x_t)
        return x_ts

    x0 = prefetch_x(0, spread=True)
    warm_src = wpool.tile([128, 128], mdt, name="warm_src")
    nc.vector.memset(warm_src, 0.25)
    warm_mv = wpool.tile([128, TC], mdt, name="warm_mv")
    nc.vector.memset(warm_mv, 0.25)
    ln_sb = {}
    for nm in ("ln1g", "ln1b"):
        t = wpool.tile([ET, NET], f32, name=nm + "_sb")
        nc.scalar.dma_start(
            out=t, in_=io[nm].rearrange("(k p) o -> p (k o)", p=ET)
        )
        ln_sb[nm] = t[:, :]
    sel_sb = wpool.tile([128, 4], f32, name="sel_sb")
    nc.scalar.dma_start(out=sel_sb, in_=io["selmask"][:])
    id_sb = wpool.tile([TS, TS], mdt, name="id_sb")
    nc.scalar.dma_start(out=id_sb, in_=io["ident"][:])
    # block "selector" for the denominator broadcast: one K=2 matmul maps
    # dr2 [2, TC] onto [128, TC] with head h's reciprocal on partitions
    # h*64..h*64+63 (host-provided constant)
    bsel = wpool.tile([2, 128], mdt, name="bsel")
    nc.scalar.dma_start(out=bsel, in_=io["bsel"][:])
    wq_sb = wpool.tile([ET, NET, DSL], mdt, name="wq_sb")
    wk_sb = wpool.tile([ET, NET, DSL], mdt, name="wk_sb")
    wv_sb = wpool.tile([ET, NET, DSL], mdt, name="wv_sb")
    nc.gpsimd.dma_start(out=wq_sb, in_=io["wq"].rearrange("(k p) d -> p k d", p=ET))
    nc.gpsimd.dma_start(out=wk_sb, in_=io["wk"].rearrange("(k p) d -> p k d", p=ET))
    nc.gpsimd.dma_start(out=wv_sb, in_=io["wv"].rearrange("(k p) d -> p k d", p=ET))
    eps_sb = wpool.tile([128, 1], f32, name="eps_sb")
    nc.vector.memset(eps_sb, EPS)
    mask_sb = wpool.tile([TS, NSUB, TC], mdt, name="mask_sb")
    wp_sb = wpool.tile([128, 2 * NDT, E], mdt, name="wp_sb")
    w1_sb = wpool.tile([ET, NET, FFN], mdt, name="w1_sb")
    w2_sb = wpool.tile([FFN + 1, E], mdt, name="w2_sb")
    b1_sb = wpool.tile([FFN, 1], f32, name="b1_sb")

    def load_late_weights():
        nc.gpsimd.dma_start(out=mask_sb, in_=io["masks"][:])
        nc.gpsimd.dma_start(
            out=wp_sb, in_=io["wp"].rearrange("(k p) d -> p k d", p=128)
        )
        nc.gpsimd.dma_start(
            out=w1_sb, in_=io["w1"].rearrange("(k p) d -> p k d", p=ET)
        )
        nc.gpsimd.dma_start(out=w2_sb, in_=io["w2e"][:])
        nc.gpsimd.dma_start(out=b1_sb, in_=io["b1"][:])
        for nm in ("ln2g", "ln2b"):
            t = wpool.tile([ET, NET], f32, name=nm + "_sb")
            nc.gpsimd.dma_start(
                out=t, in_=io[nm].rearrange("(k p) o -> p (k o)", p=ET)
            )
            ln_sb[nm] = t[:, :]

    # ---- persistent SBUF: per-chunk K^T, V(+ones), Q^T ----
    kv = pool("kv", 1)
    kT_c = [kv.tile([128, NDT, TC], mdt, name=f"kT{c}") for c in range(NTC)]
    vt_c = [kv.tile([128, NSUB, HPC, HS + 1], mdt, name=f"vt{c}")
            for c in range(NTC)]
    qT_c = [kv.tile([128, NDT, TC], mdt, name=f"qT{c}") for c in range(NTC)]

    # ---- working pools ----
    h_pool = pool("h", 6)          # [128, E] bf16: LN output rows
    mv_pool = pool("mv", 3)
    hT_pool = pool("hT", 2)        # [128, NET, TC] bf16
    pt_pool = pool("pt", 5)        # [128, 2, TC] bf16 softmax tiles
    avs_pool = pool("avs", 3)      # [HS+1, 2, TC] bf16
    dr_pool = pool("dr", 2)        # [2, TC] denominators / reciprocals
    attT_pool = pool("attT", 5)    # [128, TC] bf16
    own_pool = pool("own", 6)      # [128, HT] bf16: own-token attT columns
    stg_pool = pool("stg", 3)      # [128, 2, HT] bf16: RS staging
    peer_pool = pool("peer", 2)    # [128, NDT, HT] bf16: peer attT
    x2_pool = pool("x2", 2)        # [128, E] f32: phase-3 residual rows
    f1_pool = pool("f1", 2)
    out_pool = pool("outp", 2)
    ps_mm = pool("ps_mm", 2, space="PSUM")   # [128, 512] (1 bank each)
    ps_sc = pool("ps_sc", 2, space="PSUM")   # [128, 2, 512] (2 banks each)
    ps_av = pool("ps_av", 2, space="PSUM")   # [HS+1, 512] (1 bank each)

    # ---- HAM warm-up: ~9us of back-to-back throwaway matmuls (one PSUM
    # accumulation group -> no inter-matmul semaphores) so the PE clock gate
    # is at 8/8 and stays there until the first real QKV work arrives ----
    def warmup(n=22):
        psw = ps_mm.tile([128, TC], f32, name="psw", tag="mm")
        for i in range(n):
            nc.tensor.matmul(psw, mc(warm_src), mc(warm_mv),
                             start=(i == 0), stop=(i == n - 1))

    def layer_norm(x_t, out_t):
        """out_t (bf16) = (x - mean) * rsqrt(var + eps).
        rsqrt is exp(-0.5*ln(var+eps)) to stay in one ScalarE table set."""
        stats = mv_pool.tile([128, 2, nc.vector.BN_STATS_DIM], f32, name="stats")
        xg = x_t.rearrange("p (s q) -> p s q", s=2)
        for s in range(2):
            nc.vector.bn_stats(out=stats[:, s, :], in_=xg[:, s, :])
        mv = mv_pool.tile([128, 2], f32, name="mv")
        nc.vector.bn_aggr(out=mv, in_=stats)
        rstd = mv_pool.tile([128, 1], f32, name="rstd")
        nc.scalar.activation(
            out=rstd, in_=mv[:, 1:2], func=AF.Ln, bias=eps_sb, scale=1.0
        )
        nc.scalar.activation(out=rstd, in_=rstd, func=AF.Exp, scale=-0.5)
        nc.vector.tensor_scalar(
            out=out_t, in0=x_t, scalar1=mv[:, 0:1], scalar2=rstd,
            op0=mybir.AluOpType.subtract, op1=mybir.AluOpType.mult,
        )
        return out_t

    def transpose_cast(h_ts, g_sb, b_sb, hT, width, keep_warm=False):
        """PE-transpose len(h_ts) subtiles of h [128, E] into hT[:, k, :]
        (bf16), batching all of them into one PSUM tile per e-tile so the
        layernorm scale/bias fold costs one DVE op per [128, width].
        keep_warm: sprinkle throwaway matmuls between e-tile groups so the
        PE HAM clock stays at 8/8 while the LN1 stream trickles in."""
        nsub = len(h_ts)
        for k in range(NET):
            tp = ps_mm.tile([TS, nsub * TS], mdt, name="tp", tag="mm")
            for s in range(nsub):
                nc.tensor.transpose(
                    tp[:, s * TS:(s + 1) * TS],
                    h_ts[s][:, k * ET:(k + 1) * ET], id_sb,
                )
            if keep_warm:
                warmup(4)
            nc.vector.tensor_scalar(
                out=hT[:, k, 0:width], in0=tp,
                scalar1=g_sb[:, k:k + 1], scalar2=b_sb[:, k:k + 1],
                op0=mybir.AluOpType.mult, op1=mybir.AluOpType.add,
            )

    # =====================================================================
    # Phase 1: LN1 + transpose + QKV per chunk
    # =====================================================================
    def ln1_from(x_ts):
        h_ts = []
        for x_t in x_ts:
            h_t = h_pool.tile([128, E], mdt, name="h_t")
            h_ts.append(layer_norm(x_t, h_t))
        return h_ts

    def qkv_hT(h_ts, keep_warm=False):
        hT = hT_pool.tile([ET, NET, TC], mdt, name="hT")
        transpose_cast(h_ts, ln_sb["ln1g"], ln_sb["ln1b"], hT, TC,
                       keep_warm=keep_warm)
        return hT

    def qk_group(c, hT, dd, w_sb, dst):
        ps = ps_mm.tile([128, TC], f32, name="ps_qk", tag="mm")
        for k in range(NET):
            nc.tensor.matmul(
                ps, mc(w_sb[:, k, dd * 128:(dd + 1) * 128]),
                mc(hT[:, k, :]),
                start=(k == 0), stop=(k == NET - 1),
            )
        nc.vector.tensor_copy(dst[:, dd, :], ps)

    def v_group(c, hT, s):
        ps = ps_mm.tile([128, DSL], f32, name="ps_v", tag="mm")
        for k in range(NET):
            nc.tensor.matmul(
                ps, mc(hT[:, k, s * TS:(s + 1) * TS]), mc(wv_sb[:, k, :]),
                start=(k == 0), stop=(k == NET - 1),
            )
        nc.vector.tensor_copy(
            vt_c[c][:, s, :, 0:HS],
            ps.rearrange("p (h d) -> p h d", h=HPC),
        )
        nc.vector.memset(vt_c[c][:, s, :, HS:HS + 1], 1.0)

    def qkv_mms(c, hT):
        for dd in range(NDT):
            qk_group(c, hT, dd, wq_sb, qT_c[c])
            qk_group(c, hT, dd, wk_sb, kT_c[c])
        for s in range(NSUB):
            v_group(c, hT, s)

    def qkv_tasks(c):
        """Chunk c's LN1+transpose+QKV as a list of small closures, popped
        one-or-two per attention iteration of chunk c-1 so the PE stream
        stays dense (and the HAM clock warm) through the ScalarE-bound
        attention phase."""
        x_ts = prefetch_x(c)
        h_ts = []
        hT = hT_pool.tile([ET, NET, TC], mdt, name="hT")
        tasks = []

        def ln_task(s):
            def f():
                h_t = h_pool.tile([128, E], mdt, name="h_t")
                h_ts.append(layer_norm(x_ts[s], h_t))
            return f

        def tp_task(k):
            def f():
                tp = ps_mm.tile([TS, NSUB * TS], mdt, name="tp", tag="mm")
                for s in range(NSUB):
                    nc.tensor.transpose(
                        tp[:, s * TS:(s + 1) * TS],
                        h_ts[s][:, k * ET:(k + 1) * ET], id_sb,
                    )
                nc.vector.tensor_scalar(
                    out=hT[:, k, :], in0=tp,
                    scalar1=ln_sb["ln1g"][:, k:k + 1],
                    scalar2=ln_sb["ln1b"][:, k:k + 1],
                    op0=mybir.AluOpType.mult, op1=mybir.AluOpType.add,
                )
            return f

        for s in range(NSUB):
            tasks.append(ln_task(s))
        for k in range(NET):
            tasks.append(tp_task(k))
        for dd in range(NDT):
            tasks.append(lambda dd=dd: qk_group(c, hT, dd, wq_sb, qT_c[c]))
            tasks.append(lambda dd=dd: qk_group(c, hT, dd, wk_sb, kT_c[c]))
        for s in range(NSUB):
            tasks.append(lambda s=s: v_group(c, hT, s))
        return tasks

    # =====================================================================
    # Phase 2: attention + attT normalize + pair exchange
    # =====================================================================
    def finish_pair(av_sb, dr2, attT):
        """Broadcast the reciprocal denominators across each head's 64
        partitions with a K=2 selector-matmul and normalize into attT."""
        rb = ps_mm.tile([128, TC], f32, name="rb", tag="mm")
        nc.tensor.matmul(rb, mc(bsel), mc(dr2), start=True, stop=True)
        for hh in range(2):
            nc.vector.tensor_mul(
                attT[hh * HS:(hh + 1) * HS, :], av_sb[0:HS, hh, :],
                rb[hh * HS:(hh + 1) * HS, :],
            )

    def stage_pair(c, pr, attT):
        """Write attT's two token-half column blocks to the RS staging DRAM
        (own-dest slot zeroed via selmask) and produce the own-token columns
        for the local proj."""
        stg = stg_pool.tile([128, 2, HT], mdt, name="stg")
        for j in range(2):
            nc.vector.tensor_scalar_mul(
                stg[:, j, :], attT[:, j * HT:(j + 1) * HT],
                sel_sb[:, j:j + 1],
            )
        if c == NTC - 1:
            dst = agi_3[pr // 2][:, pr % 2, :, :]
        else:
            dst = agi_c[c][:, pr, :, :]
        nc.sync.dma_start(out=dst.rearrange("j p f -> p j f"), in_=stg)
        # own-token columns: attT[:, g*256:(g+1)*256] selected via input data
        ow = own_pool.tile([128, HT], mdt, name="ow")
        tmp = stg_pool.tile([128, HT], mdt, name="owt", tag="owt")
        nc.vector.tensor_scalar_mul(tmp, attT[:, 0:HT], sel_sb[:, 2:3])
        nc.vector.tensor_scalar_mul(ow, attT[:, HT:2 * HT], sel_sb[:, 3:4])
        nc.vector.tensor_add(ow, ow, tmp)
        return ow

    def rs_chunk(c, half=None):
        if c == NTC - 1:
            nc.gpsimd.collective_compute(
                "ReduceScatter", mybir.AluOpType.add, replica_groups=PAIRS,
                ins=[agi_3[half][:]], outs=[ago_3[half][:]],
            )
        else:
            nc.gpsimd.collective_compute(
                "ReduceScatter", mybir.AluOpType.add, replica_groups=PAIRS,
                ins=[agi_c[c][:]], outs=[ago_c[c][:]],
            )

    def attention_chunk(c, fillers=None, micro=None):
        nkt = (c + 1) * NSUB
        owns = []
        pending = None
        micro = micro or []
        iters_left = [NDT * nkt]

        def pop_micro():
            if micro:
                npop = -(-len(micro) // max(1, iters_left[0]))
                for _ in range(npop):
                    micro.pop(0)()
            iters_left[0] -= 1

        def finish_stage(pending, pr_done):
            finish_pair(*pending)
            owns.append(stage_pair(c, pr_done, pending[2]))
            if c == NTC - 1 and pr_done == 1:
                rs_chunk(c, half=0)
            if pr_done == NDT - 1:
                rs_chunk(c, half=1 if c == NTC - 1 else None)

        for pr in range(NDT):  # head pair = d-tile
            fn = (fillers or {}).pop(pr, None)
            if fn is not None:
                fn()
            av_ps = [ps_av.tile([HS + 1, TC], f32, name="avp") for _ in range(2)]
            def av_mms(pi, ppt, p0, last):
                # columns below p0 get no contribution from this t_k tile
                # (fully above the diagonal); PSUM accumulation is
                # per-element so the shorter matmul leaves them untouched
                for hh in range(2):
                    nc.tensor.matmul(
                        av_ps[hh][:, p0:TC],
                        mc(vt_c[pi // NSUB][:, pi % NSUB, pr * 2 + hh, :]),
                        mc(ppt[:, hh, p0:TC]),
                        start=(pi == 0), stop=last,
                    )

            avq = []  # stagger AV matmuls 2 units behind exp+mask
            for i in range(nkt):
                m = i - c * NSUB
                # p0: first t_q column this t_k tile can attend to
                p0 = m * TS if m > 0 else 0
                sc2 = ps_sc.tile([TS, 2, TC], f32, name="sc2")
                for hh in range(2):
                    h0 = hh * HS
                    nc.tensor.matmul(
                        sc2[:, hh, p0:TC],
                        mc(kT_c[i // NSUB][h0:h0 + HS, pr,
                                           (i % NSUB) * TS:(i % NSUB + 1) * TS]),
                        mc(qT_c[c][h0:h0 + HS, pr, p0:TC]),
                        start=True, stop=True,
                    )
                pt2 = pt_pool.tile([TS, 2, TC], mdt, name="pt2")
                nc.scalar.activation(
                    out=pt2[:, :, p0:TC], in_=sc2[:, :, p0:TC],
                    func=AF.Exp, scale=SCALE,
                )
                if m >= 0:
                    # diagonal TS block: zero t_k > t_q within it
                    for hh in range(2):
                        nc.vector.tensor_mul(
                            pt2[:, hh, p0:p0 + TS], pt2[:, hh, p0:p0 + TS],
                            mask_sb[:, m, p0:p0 + TS],
                        )
                avq.append((i, pt2, p0))
                if len(avq) > 2:
                    av_mms(*avq.pop(0), last=False)
                if i == 3 and pending is not None:
                    finish_stage(pending, pr - 1)
                    pending = None
                pop_micro()
            while avq:
                av_mms(*avq.pop(0), last=(len(avq) == 0))
            av_sb = avs_pool.tile([HS + 1, 2, TC], mdt, name="av_sb")
            for hh in range(2):
                nc.vector.tensor_copy(av_sb[:, hh, :], av_ps[hh])
            # 1/den on ScalarE as exp(-ln(den)); the two heads' denominator
            # rows are DMA-gathered onto two partitions first so the
            # activation runs on 2 lanes instead of 1
            den2 = dr_pool.tile([2, TC], mdt, name="den2", tag="den")
            for hh in range(2):
                nc.sync.dma_start(
                    out=den2[hh:hh + 1, :], in_=av_sb[HS:HS + 1, hh, :]
                )
            lden = dr_pool.tile([2, TC], f32, name="lden", tag="lden")
            nc.scalar.activation(out=lden, in_=den2, func=AF.Ln)
            dr2 = dr_pool.tile([2, TC], mdt, name="dr2", tag="dr")
            nc.scalar.activation(out=dr2, in_=lden, func=AF.Exp, scale=-1.0)
            attT = attT_pool.tile([128, TC], mdt, name="attT")
            if pending is not None:
                finish_stage(pending, pr - 1)
            pending = (av_sb, dr2, attT)
        while micro:
            micro.pop(0)()
        finish_stage(pending, NDT - 1)
        return owns

    # =====================================================================
    # Phase 3: peer attT + proj + residual + LN2 + FFN on own 256 tokens
    # =====================================================================
    def tail_chunk(c, owns, peer_waits=None):
        """proj for this core's 256-token shard (contraction over own 4 +
        peer 4 d-tiles), then residual + LN2 + FFN + output."""
        # peer/x_own/out go on the gpsimd DMA queue: it carries the
        # collectives, so the peer load orders naturally behind its RS and
        # none of these (which can wait multi-us on data) block the sync
        # queue that carries the attention-critical den/stage transfers
        peer = peer_pool.tile([128, NDT, HT], mdt, name="peer")
        if c == NTC - 1:
            for h in range(2):
                nc.gpsimd.dma_start(
                    out=peer[:, 2 * h:2 * h + 2, :],
                    in_=ago_3[h].rearrange("d p f -> p d f"),
                )
        else:
            nc.gpsimd.dma_start(
                out=peer, in_=ago_c[c].rearrange("d p f -> p d f")
            )
        x2_ts = []
        h2_ts = []
        for s in range(2):
            x2_t = x2_pool.tile([128, E], f32, name="x2_t")
            nc.sync.dma_start(
                out=x2_t, in_=io["x_own"][c, s * TS:(s + 1) * TS, :]
            )
            for n in range(2):
                ps = ps_mm.tile([128, TC], f32, name="ps_pr", tag="mm")
                for dd in range(NDT):
                    nc.tensor.matmul(
                        ps, mc(owns[dd][:, s * TS:(s + 1) * TS]),
                        mc(wp_sb[:, dd, n * TC:(n + 1) * TC]),
                        start=(dd == 0), stop=False,
                    )
                for dd in range(NDT):
                    nc.tensor.matmul(
                        ps, mc(peer[:, dd, s * TS:(s + 1) * TS]),
                        mc(wp_sb[:, NDT + dd, n * TC:(n + 1) * TC]),
                        start=False, stop=(dd == NDT - 1),
                    )
                nc.vector.tensor_add(
                    x2_t[:, n * TC:(n + 1) * TC], x2_t[:, n * TC:(n + 1) * TC],
                    ps,
                )
            x2_ts.append(x2_t)
            h2_t = h_pool.tile([128, E], mdt, name="h2_t", tag="h_t")
            layer_norm(x2_t, h2_t)
            h2_ts.append(h2_t)
        h2T = hT_pool.tile([ET, NET, HT], mdt, name="h2T")
        transpose_cast(h2_ts, ln_sb["ln2g"], ln_sb["ln2b"], h2T, HT)
        f1 = f1_pool.tile([FFN + 1, HT], mdt, name="f1")
        nc.vector.memset(f1, 1.0)  # row FFN stays 1.0 (b2 matmul row)
        ps_f = ps_mm.tile([FFN, HT], f32, name="ps_f", tag="mm")
        for k in range(NET):
            nc.tensor.matmul(
                ps_f, mc(w1_sb[:, k, :]), mc(h2T[:, k, :]),
                start=(k == 0), stop=(k == NET - 1),
            )
        nc.scalar.activation(
            out=f1[0:FFN, :], in_=ps_f, func=AF.Relu, bias=b1_sb, scale=1.0
        )
        for s in range(2):
            o_t = out_pool.tile([128, E], f32, name="o_t")
            for n in range(2):
                ps = ps_mm.tile([128, TC], f32, name="ps_o", tag="mm")
                nc.tensor.matmul(
                    ps, mc(f1[:, s * TS:(s + 1) * TS]),
                    mc(w2_sb[:, n * TC:(n + 1) * TC]),
                    start=True, stop=True,
                )
                nc.vector.tensor_add(
                    o_t[:, n * TC:(n + 1) * TC], ps,
                    x2_ts[s][:, n * TC:(n + 1) * TC],
                )
            nc.sync.dma_start(out=out[c, s * TS:(s + 1) * TS, :], in_=o_t)

    # ---- schedule: QKV chunk 0 up front, then attention chunk c with
    # QKV chunk c+1 micro-interleaved into its iteration stream and chunk
    # c-1's tail (peer+proj+FFN) as a filler at pair 2 ----
    warmup(12)
    load_late_weights()
    qkv_mms(0, qkv_hT(ln1_from(x0), keep_warm=True))
    micro = qkv_tasks(1)
    pending_tail = None
    for c in range(NTC):
        fillers = {}
        if pending_tail is not None:
            fillers[2] = pending_tail
        owns = attention_chunk(c, fillers=fillers, micro=micro)
        micro = qkv_tasks(c + 2) if c + 2 < NTC else []
        pending_tail = (lambda cc, oo: lambda: tail_chunk(cc, oo))(c, owns)
    pending_tail()


# =========================================================================
# Host side
# =========================================================================
def _make_masks(np_mdt):
    # masks[p, d, f] = 1 iff t_k <= t_q for the diagonal block at offset d,
    # i.e. f >= 128*d + p  (t_k = 128*i + p, t_q = 512*c + f, i = 4*c + d)
    m = np.zeros((TS, NSUB, TC), dtype=np.float32)
    for d in range(NSUB):
        for p in range(TS):
            m[p, d, d * TS + p:] = 1.0
    return m.astype(np_mdt)


_NC_CACHE = {}
RUN_KWARGS = {}      # test harness may set {"trace": True} for profiling
LAST_RESULT = None   # BassKernelResults of the most recent run


def kernel(x, wq, wk, wv, w_proj, b_proj, w1, b1, w2, b2, ln1_g, ln1_b, ln2_g,
           ln2_b):
    mode = MM_MODE
    np_mdt = _np_mdt(mode)
    if mode not in _NC_CACHE:
        _NC_CACHE[mode] = build(mode)
    nc = _NC_CACHE[mode]

    x = np.asarray(x, np.float32)
    bp = np.asarray(b_proj, np.float32)
    masks = _make_masks(np_mdt)
    identity = np.eye(TS, dtype=np.float32)
    w2e = np.concatenate([np.asarray(w2, np.float32),
                          np.asarray(b2, np.float32)[None, :]], axis=0)
    wp_full = np.asarray(w_proj, np.float32)
    bsel_np = np.zeros((2, TS), np.float32)
    bsel_np[0, 0:HS] = 1.0
    bsel_np[1, HS:TS] = 1.0

    def own_rows(c, g):
        return np.r_[c * TC + g * HT:c * TC + (g + 1) * HT]

    ln1g = np.asarray(ln1_g, np.float32)
    ln1b = np.asarray(ln1_b, np.float32)
    in_maps = []
    for core in range(NCORE):
        b, g = core // 2, core % 2
        sl = slice(g * DSL, (g + 1) * DSL)
        slp = slice((1 - g) * DSL, (2 - g) * DSL)
        x_own = np.stack(
            [x[b, own_rows(c, g), :] for c in range(NTC)]
        ) + bp[None, None, :]
        selmask = np.zeros((128, 4), np.float32)
        selmask[:, 0] = 0.0 if g == 0 else 1.0   # stage: zero own dest slot
        selmask[:, 1] = 0.0 if g == 1 else 1.0
        selmask[:, 2] = 1.0 if g == 0 else 0.0   # own-token half select
        selmask[:, 3] = 1.0 if g == 1 else 0.0
        wp_core = np.concatenate([wp_full[sl, :], wp_full[slp, :]], axis=0)
        in_maps.append({
            "x": x[b].astype(np_mdt),
            "x_own": x_own,
            "selmask": selmask,
            "wq": np.asarray(wq, np.float32)[:, sl].astype(np_mdt),
            "wk": np.asarray(wk, np.float32)[:, sl].astype(np_mdt),
            "wv": np.asarray(wv, np.float32)[:, sl].astype(np_mdt),
            "wp": wp_core.astype(np_mdt),
            "w1": np.asarray(w1, np.float32).astype(np_mdt),
            "w2e": w2e.astype(np_mdt),
            "b1": np.asarray(b1, np.float32)[:, None],
            "ln1g": np.asarray(ln1_g, np.float32)[:, None],
            "ln1b": np.asarray(ln1_b, np.float32)[:, None],
            "ln2g": np.asarray(ln2_g, np.float32)[:, None],
            "ln2b": np.asarray(ln2_b, np.float32)[:, None],
            "masks": masks,
            "ident": identity.astype(np_mdt),
            "bsel": bsel_np.astype(np_mdt),
        })
    global LAST_RESULT
    res = run_bass_kernel_spmd(nc, in_maps, list(range(NCORE)), **RUN_KWARGS)
    LAST_RESULT = res
    outp = np.empty((B, T, E), np.float32)
    for core in range(NCORE):
        b, g = core // 2, core % 2
        o = res.results[core]["out"]
        for c in range(NTC):
            outp[b, own_rows(c, g), :] = o[c]
    return outp


# revision 52
# speedup vs baseline: 1.2093x; 1.0806x over previous
"""Trainium2 Bass kernel: pre-LN transformer block (B=4, T=2048, E=1024, H=16, FFN=100).

Sharding (8 NeuronCores): core 2b+g handles batch b, head-group g (8 of 16
heads, i.e. a 512-wide slice of the QKV output dim / proj input dim).  Both
cores of a pair compute attention for all 2048 tokens of their batch; after
each chunk's attention the pair exchanges normalized attention outputs
(attT) for the tokens the *other* core owns via a zero-masked pair
ReduceScatter (each core's contribution to its own slot is multiplied by a
per-core 0/1 input mask, so the RS-add delivers exactly the peer's attT) —
256KB on the wire per chunk instead of the 1MB proj-partial RS, and it fires
*before* proj, so proj + residual + LN2 + FFN for the core's own 256-token
shard run with no collective behind them.  All rank-dependent choices (which
token half is "mine", the proj weight row order, residual rows) live in
per-core input data, keeping the single SPMD program rank-symmetric.

Schedule: a burst of throwaway warm-up matmuls at t~0 flips the PE HAM clock
gate to 8/8 before real work lands; chunk 0's x subtiles and wq stream first
so LN1+QKV start ~15us in.  All four chunks' LN1+QKV run first (dense PE
work), then attention per chunk; chunk c's tail (peer attT load + proj + FFN)
is emitted inside chunk c+1's attention stream so the PE never idles and
every RS overlaps later attention; the last chunk's RS is split in two so
its tail starts earlier.  x is loaded in bf16 (it only feeds LN1); the f32
residual rows arrive separately as x_own (with b_proj folded in host-side).

Attention: scores are computed transposed, S^T[t_k, t_q] = k^T.T @ q^T, with
q^T/k^T in [head_dim, token] layout (from PE-transposed bf16 LN output).  The
two heads of a d-tile pair occupy partitions 0-63 / 64-127 and run as
concurrent row-group matmuls into one 2-bank PSUM tile, so a single ScalarE
exp (1/sqrt(E) scale folded in) covers both.  Diagonal t_k tiles compute only
the causally live columns (shorter score/AV matmuls + sliced exp) and one
TS-wide mask multiply; AV matmuls trail the exp stream by two tiles.  The
softmax denominator comes from a ones column appended to V; its reciprocal is
exp(-ln(den)) on ScalarE after a tiny SBUF->SBUF DMA gathers both heads'
denominator rows onto two partitions (a [1, N] activation would serialize on
one lane), and is broadcast across the head's 64 partitions with a K=1
ones-matmul into PSUM.  LayerNorm rsqrt is exp(-0.5*ln(var+eps)), keeping
the whole kernel on a single ScalarE table set (natural_log_exp_and_others).
"""

from contextlib import ExitStack

import numpy as np
import ml_dtypes

import concourse.bass as bass
import concourse.mybir as mybir
import concourse.tile as tile
from concourse.bass_utils import run_bass_kernel_spmd
from concourse.vector_clock import ScopedClock


class SplitDrainTC(tile.TileContext):
    """Works around a walrus codegen limit: an SP CTRL instruction may carry
    only one sync wait, so the kernel-tail drain's waits are split onto
    preceding single-wait nops."""

    def _drain_and_barrier(self, tick_clock, wait_clock):
        probe = self.nc.sync.nop(nofuse=True)
        wait_clock.add_sem_waits(
            probe.ins, ScopedClock({None: tick_clock.global_clock})
        )
        si = probe.ins.sync_info
        waits = list(si.on_wait) if si is not None else []
        if len(waits) > 1:
            si.on_wait = [waits[0]]
            for w in waits[1:]:
                n2 = self.nc.sync.nop(nofuse=True)
                n2.ins.sync_info = mybir.SyncInfo(on_wait=[w], on_update=[])
        self.nc.sync.drain()
        self.nc.all_engine_barrier()
        popped = self.nc._tile_sem_poison_stack.pop()
        assert popped is self._sem_poison
        self.nc.clear_and_free_semaphores(list(self.sems.allocated().values()))
        self.nc.all_engine_barrier()

B, T, E, H, HS, FFN = 4, 2048, 1024, 16, 64, 100
EPS = 1e-5
NCORE = 8
TC = 512            # token chunk
NTC = T // TC       # 4
TS = 128            # token subtile
NSUB = TC // TS     # 4
ET = 128            # embed tile
NET = E // ET       # 8
DSL = E // 2        # per-core qkv output slice (8 heads * 64)
NDT = DSL // 128    # 4 d-tiles (2 heads each)
HPC = H // 2        # 8 heads per core
HT = TC // 2        # 256: tokens owned per core per chunk
SCALE = float(E) ** -0.5
PAIRS = [[0, 1], [2, 3], [4, 5], [6, 7]]

MM_MODE = "bf16"    # "bf16" | "f32r" | "f32"
AF = mybir.ActivationFunctionType


def _mdt(mode):
    return mybir.dt.bfloat16 if mode == "bf16" else mybir.dt.float32


def _np_mdt(mode):
    return ml_dtypes.bfloat16 if mode == "bf16" else np.float32


def build(mode=MM_MODE):
    f32 = mybir.dt.float32
    mdt = _mdt(mode)

    def mc(ap):
        """Cast an AP for use as a matmul operand."""
        if mode == "f32r":
            return ap.bitcast(mybir.dt.float32r)
        return ap

    nc = bass.Bass(num_devices=NCORE)

    io = {}

    def param(name, shape, dtype):
        io[name] = nc.declare_dram_parameter(name, shape, dtype, isOutput=False)

    param("x", [T, E], mdt)                  # bf16: only feeds LN1
    param("x_own", [NTC, HT, E], f32)        # own residual rows, + b_proj
    # cols 0-1: RS staging sel (0 for my own dest slot); cols 2-3: own-token
    # half select (1 for my half)
    param("selmask", [128, 4], f32)
    param("wq", [E, DSL], mdt)
    param("wk", [E, DSL], mdt)
    param("wv", [E, DSL], mdt)
    param("wp", [E, E], mdt)                 # rows reordered: [own 512; peer 512]
    param("w1", [E, FFN], mdt)
    param("w2e", [FFN + 1, E], mdt)    # w2 with b2 as the extra last row
    param("b1", [FFN, 1], f32)
    param("ln1g", [E, 1], f32)
    param("ln1b", [E, 1], f32)
    param("ln2g", [E, 1], f32)
    param("ln2b", [E, 1], f32)
    param("masks", [TS, NSUB, TC], mdt)
    param("ident", [TS, TS], mdt)
    param("bsel", [2, TS], mdt)
    io["out"] = nc.declare_dram_parameter(
        "out", [NTC, HT, E], f32, isOutput=True
    )

    with SplitDrainTC(nc) as tc:
        with ExitStack() as ctx:
            _build_tile(ctx, tc, nc, mode, mdt, f32, mc, io)
    _split_waits(nc)
    return nc


def _split_waits(nc, maxw=1):
    """walrus codegen accepts a limited number of sync waits per instruction;
    move the excess onto same-engine NoOps inserted just before."""
    import bass_rust
    n = 0
    for f in nc.m.functions:
        for b in f.blocks:
            new = []
            for inst in b.instructions:
                si = inst.sync_info
                # fixed-length ISA instructions can't carry waits at all
                cap = 0 if isinstance(inst, bass_rust.InstISA) else maxw
                if si is not None and len(si.on_wait) > cap:
                    waits = list(si.on_wait)
                    keep = waits[-cap:] if cap else []
                    excess = waits[:-cap] if cap else waits
                    for w in excess:
                        nop = mybir.InstNoOp(
                            name=f"{inst.name}-wsplit{n}", engine=inst.engine
                        )
                        nop.bass_nofuse = True
                        n += 1
                        nop.sync_info = mybir.SyncInfo(
                            on_wait=[w], on_update=[]
                        )
                        new.append(nop)
                    si.on_wait = keep
                new.append(inst)
            if n:
                b.instructions = new


def _build_tile(ctx, tc, nc, mode, mdt, f32, mc, io):
    x, out = io["x"], io["out"]

    def pool(name, bufs, space="SBUF"):
        return ctx.enter_context(tc.tile_pool(name=name, bufs=bufs, space=space))

    # ---- internal DRAM: per-chunk attT-exchange RS buffers.  agi[j] holds
    # this core's attT columns for the tokens rank j owns (own-dest slot
    # zero-masked); the pair RS-add delivers the peer's attT for my tokens.
    dram = pool("dram", 1, space="DRAM")
    al_i = dram.tile([2, 128], mybir.dt.float32, name="al_i")
    al_o = dram.tile([1, 128], mybir.dt.float32, name="al_o")
    agi_c = [dram.tile([2, NDT, TS, HT], mdt, name=f"agi{c}") for c in range(3)]
    ago_c = [dram.tile([NDT, TS, HT], mdt, name=f"ago{c}") for c in range(3)]
    # last chunk: two half-exchanges (d-tiles 0-1 / 2-3) so its tail starts
    # as soon as the first half's attention pairs finish
    agi_3 = [dram.tile([2, 2, TS, HT], mdt, name=f"agi3{h}") for h in range(2)]
    ago_3 = [dram.tile([2, TS, HT], mdt, name=f"ago3{h}") for h in range(2)]

    # ---- persistent SBUF: weights & constants.  Emission order sets the
    # tile scheduler's priority: chunk 0's x subtiles + the small consts +
    # wq go first so LN1+QKV start early. ----
    wpool = pool("weights", 1)
    xt_pool = pool("xt", 4)        # [128, E] bf16: x rows for LN1

    def prefetch_x(c, spread=False):
        qs = [nc.sync, nc.scalar, nc.scalar, nc.sync] if spread \
            else [nc.sync] * NSUB
        x_ts = []
        for s in range(NSUB):
            r0 = c * TC + s * TS
            x_t = xt_pool.tile([128, E], mdt, name="x_t")
            qs[s].dma_start(out=x_t, in_=x[r0:r0 + TS, :])
            x_ts.append(x_t)
        return x_ts

    x0 = prefetch_x(0, spread=True)
    warm_src = wpool.tile([128, 128], mdt, name="warm_src")
    nc.vector.memset(warm_src, 0.25)
    warm_mv = wpool.tile([128, TC], mdt, name="warm_mv")
    nc.vector.memset(warm_mv, 0.25)
    ln_sb = {}
    for nm in ("ln1g", "ln1b"):
        t = wpool.tile([ET, NET], f32, name=nm + "_sb")
        nc.scalar.dma_start(
            out=t, in_=io[nm].rearrange("(k p) o -> p (k o)", p=ET)
        )
        ln_sb[nm] = t[:, :]
    sel_sb = wpool.tile([128, 4], f32, name="sel_sb")
    nc.scalar.dma_start(out=sel_sb, in_=io["selmask"][:])
    id_sb = wpool.tile([TS, TS], mdt, name="id_sb")
    nc.scalar.dma_start(out=id_sb, in_=io["ident"][:])
    # block "selector" for the denominator broadcast: one K=2 matmul maps
    # dr2 [2, TC] onto [128, TC] with head h's reciprocal on partitions
    # h*64..h*64+63 (host-provided constant)
    bsel = wpool.tile([2, 128], mdt, name="bsel")
    nc.scalar.dma_start(out=bsel, in_=io["bsel"][:])
    wq_sb = wpool.tile([ET, NET, DSL], mdt, name="wq_sb")
    wk_sb = wpool.tile([ET, NET, DSL], mdt, name="wk_sb")
    wv_sb = wpool.tile([ET, NET, DSL], mdt, name="wv_sb")
    nc.gpsimd.dma_start(out=wq_sb, in_=io["wq"].rearrange("(k p) d -> p k d", p=ET))
    nc.gpsimd.dma_start(out=wk_sb, in_=io["wk"].rearrange("(k p) d -> p k d", p=ET))
    nc.gpsimd.dma_start(out=wv_sb, in_=io["wv"].rearrange("(k p) d -> p k d", p=ET))
    eps_sb = wpool.tile([128, 1], f32, name="eps_sb")
    nc.vector.memset(eps_sb, EPS)
    mask_sb = wpool.tile([TS, NSUB, TC], mdt, name="mask_sb")
    wp_sb = wpool.tile([128, 2 * NDT, E], mdt, name="wp_sb")
    w1_sb = wpool.tile([ET, NET, FFN], mdt, name="w1_sb")
    w2_sb = wpool.tile([FFN + 1, E], mdt, name="w2_sb")
    b1_sb = wpool.tile([FFN, 1], f32, name="b1_sb")

    def load_late_weights():
        nc.gpsimd.dma_start(out=mask_sb, in_=io["masks"][:])
        nc.gpsimd.dma_start(
            out=wp_sb, in_=io["wp"].rearrange("(k p) d -> p k d", p=128)
        )
        nc.gpsimd.dma_start(
            out=w1_sb, in_=io["w1"].rearrange("(k p) d -> p k d", p=ET)
        )
        nc.gpsimd.dma_start(out=w2_sb, in_=io["w2e"][:])
        nc.gpsimd.dma_start(out=b1_sb, in_=io["b1"][:])
        for nm in ("ln2g", "ln2b"):
            t = wpool.tile([ET, NET], f32, name=nm + "_sb")
            nc.gpsimd.dma_start(
                out=t, in_=io[nm].rearrange("(k p) o -> p (k o)", p=ET)
            )
            ln_sb[nm] = t[:, :]

    # ---- persistent SBUF: per-chunk K^T, V(+ones), Q^T ----
    kv = pool("kv", 1)
    kT_c = [kv.tile([128, NDT, TC], mdt, name=f"kT{c}") for c in range(NTC)]
    vt_c = [kv.tile([128, NSUB, HPC, HS + 1], mdt, name=f"vt{c}")
            for c in range(NTC)]
    qT_c = [kv.tile([128, NDT, TC], mdt, name=f"qT{c}") for c in range(NTC)]

    # ---- working pools ----
    h_pool = pool("h", 6)          # [128, E] bf16: LN output rows
    mv_pool = pool("mv", 3)
    hT_pool = pool("hT", 2)        # [128, NET, TC] bf16
    pt_pool = pool("pt", 5)        # [128, 2, TC] bf16 softmax tiles
    avs_pool = pool("avs", 3)      # [HS+1, 2, TC] bf16
    dr_pool = pool("dr", 2)        # [2, TC] denominators / reciprocals
    attT_pool = pool("attT", 5)    # [128, TC] bf16
    own_pool = pool("own", 6)      # [128, HT] bf16: own-token attT columns
    stg_pool = pool("stg", 3)      # [128, 2, HT] bf16: RS staging
    peer_pool = pool("peer", 2)    # [128, NDT, HT] bf16: peer attT
    x2_pool = pool("x2", 2)        # [128, E] f32: phase-3 residual rows
    f1_pool = pool("f1", 2)
    out_pool = pool("outp", 2)
    ps_mm = pool("ps_mm", 2, space="PSUM")   # [128, 512] (1 bank each)
    ps_sc = pool("ps_sc", 2, space="PSUM")   # [128, 2, 512] (2 banks each)
    ps_av = pool("ps_av", 2, space="PSUM")   # [HS+1, 512] (1 bank each)

    # ---- HAM warm-up: ~9us of back-to-back throwaway matmuls (one PSUM
    # accumulation group -> no inter-matmul semaphores) so the PE clock gate
    # is at 8/8 and stays there until the first real QKV work arrives ----
    def warmup(n=22):
        psw = ps_mm.tile([128, TC], f32, name="psw", tag="mm")
        for i in range(n):
            nc.tensor.matmul(psw, mc(warm_src), mc(warm_mv),
                             start=(i == 0), stop=(i == n - 1))

    def layer_norm(x_t, out_t):
        """out_t (bf16) = (x - mean) * rsqrt(var + eps).
        rsqrt is exp(-0.5*ln(var+eps)) to stay in one ScalarE table set."""
        stats = mv_pool.tile([128, 2, nc.vector.BN_STATS_DIM], f32, name="stats")
        xg = x_t.rearrange("p (s q) -> p s q", s=2)
        for s in range(2):
            nc.vector.bn_stats(out=stats[:, s, :], in_=xg[:, s, :])
        mv = mv_pool.tile([128, 2], f32, name="mv")
        nc.vector.bn_aggr(out=mv, in_=stats)
        rstd = mv_pool.tile([128, 1], f32, name="rstd")
        nc.scalar.activation(
            out=rstd, in_=mv[:, 1:2], func=AF.Ln, bias=eps_sb, scale=1.0
        )
        nc.scalar.activation(out=rstd, in_=rstd, func=AF.Exp, scale=-0.5)
        nc.vector.tensor_scalar(
            out=out_t, in0=x_t, scalar1=mv[:, 0:1], scalar2=rstd,
            op0=mybir.AluOpType.subtract, op1=mybir.AluOpType.mult,
        )
        return out_t

    def transpose_cast(h_ts, g_sb, b_sb, hT, width):
        """Transpose len(h_ts) subtiles of h [128, E] into hT[:, k, :]
        (bf16), batching all of them into one PSUM tile per e-tile so the
        layernorm scale/bias fold costs one DVE op per [128, width].
        The transpose is a *regular* matmul against the identity (out =
        h.T @ I): transpose-mode matmuls cost ~275ns each (SBUF access
        latency, and they don't count as HAM activity); normal-mode runs
        ~107ns and keeps the clock gate warm."""
        nsub = len(h_ts)
        for k in range(NET):
            tp = ps_mm.tile([TS, nsub * TS], f32, name="tp", tag="mm")
            for s in range(nsub):
                nc.tensor.matmul(
                    tp[:, s * TS:(s + 1) * TS],
                    mc(h_ts[s][:, k * ET:(k + 1) * ET]), mc(id_sb),
                    start=True, stop=True,
                )
            nc.vector.tensor_scalar(
                out=hT[:, k, 0:width], in0=tp,
                scalar1=g_sb[:, k:k + 1], scalar2=b_sb[:, k:k + 1],
                op0=mybir.AluOpType.mult, op1=mybir.AluOpType.add,
            )

    # =====================================================================
    # Phase 1: LN1 + transpose + QKV per chunk
    # =====================================================================
    def ln1_from(x_ts):
        h_ts = []
        for x_t in x_ts:
            h_t = h_pool.tile([128, E], mdt, name="h_t")
            h_ts.append(layer_norm(x_t, h_t))
        return h_ts

    def qkv_hT(h_ts):
        hT = hT_pool.tile([ET, NET, TC], mdt, name="hT")
        transpose_cast(h_ts, ln_sb["ln1g"], ln_sb["ln1b"], hT, TC)
        return hT

    def qk_group(c, hT, dd, w_sb, dst):
        ps = ps_mm.tile([128, TC], f32, name="ps_qk", tag="mm")
        for k in range(NET):
            nc.tensor.matmul(
                ps, mc(w_sb[:, k, dd * 128:(dd + 1) * 128]),
                mc(hT[:, k, :]),
                start=(k == 0), stop=(k == NET - 1),
            )
        nc.vector.tensor_copy(dst[:, dd, :], ps)

    def v_group(c, hT, s):
        ps = ps_mm.tile([128, DSL], f32, name="ps_v", tag="mm")
        for k in range(NET):
            nc.tensor.matmul(
                ps, mc(hT[:, k, s * TS:(s + 1) * TS]), mc(wv_sb[:, k, :]),
                start=(k == 0), stop=(k == NET - 1),
            )
        nc.vector.tensor_copy(
            vt_c[c][:, s, :, 0:HS],
            ps.rearrange("p (h d) -> p h d", h=HPC),
        )
        nc.vector.memset(vt_c[c][:, s, :, HS:HS + 1], 1.0)

    def qkv_mms(c, hT):
        for dd in range(NDT):
            qk_group(c, hT, dd, wq_sb, qT_c[c])
            qk_group(c, hT, dd, wk_sb, kT_c[c])
        for s in range(NSUB):
            v_group(c, hT, s)

    def qkv_tasks(c):
        """Chunk c's LN1+transpose+QKV as a list of small closures, popped
        one-or-two per attention iteration of chunk c-1 so the PE stream
        stays dense (and the HAM clock warm) through the ScalarE-bound
        attention phase."""
        x_ts = prefetch_x(c)
        h_ts = []
        hT = hT_pool.tile([ET, NET, TC], mdt, name="hT")
        tasks = []

        def ln_task(s):
            def f():
                h_t = h_pool.tile([128, E], mdt, name="h_t")
                h_ts.append(layer_norm(x_ts[s], h_t))
            return f

        def tp_task(k):
            def f():
                tp = ps_mm.tile([TS, NSUB * TS], f32, name="tp", tag="mm")
                for s in range(NSUB):
                    nc.tensor.matmul(
                        tp[:, s * TS:(s + 1) * TS],
                        mc(h_ts[s][:, k * ET:(k + 1) * ET]), mc(id_sb),
                        start=True, stop=True,
                    )
                nc.vector.tensor_scalar(
                    out=hT[:, k, :], in0=tp,
                    scalar1=ln_sb["ln1g"][:, k:k + 1],
                    scalar2=ln_sb["ln1b"][:, k:k + 1],
                    op0=mybir.AluOpType.mult, op1=mybir.AluOpType.add,
                )
            return f

        for s in range(NSUB):
            tasks.append(ln_task(s))
        for k in range(NET):
            tasks.append(tp_task(k))
        for dd in range(NDT):
            tasks.append(lambda dd=dd: qk_group(c, hT, dd, wq_sb, qT_c[c]))
            tasks.append(lambda dd=dd: qk_group(c, hT, dd, wk_sb, kT_c[c]))
        for s in range(NSUB):
            tasks.append(lambda s=s: v_group(c, hT, s))
        return tasks

    # =====================================================================
    # Phase 2: attention + attT normalize + pair exchange
    # =====================================================================
    def finish_pair(av_sb, dr2, attT):
        """Broadcast the reciprocal denominators across each head's 64
        partitions with a K=2 selector-matmul and normalize into attT."""
        rb = ps_mm.tile([128, TC], f32, name="rb", tag="mm")
        nc.tensor.matmul(rb, mc(bsel), mc(dr2), start=True, stop=True)
        for hh in range(2):
            nc.vector.tensor_mul(
                attT[hh * HS:(hh + 1) * HS, :], av_sb[0:HS, hh, :],
                rb[hh * HS:(hh + 1) * HS, :],
            )

    def stage_pair(c, pr, attT):
        """Write attT's two token-half column blocks to the RS staging DRAM
        (own-dest slot zeroed via selmask) and produce the own-token columns
        for the local proj."""
        stg = stg_pool.tile([128, 2, HT], mdt, name="stg")
        for j in range(2):
            nc.vector.tensor_scalar_mul(
                stg[:, j, :], attT[:, j * HT:(j + 1) * HT],
                sel_sb[:, j:j + 1],
            )
        if c == NTC - 1:
            dst = agi_3[pr // 2][:, pr % 2, :, :]
        else:
            dst = agi_c[c][:, pr, :, :]
        nc.sync.dma_start(out=dst.rearrange("j p f -> p j f"), in_=stg)
        # own-token columns: attT[:, g*256:(g+1)*256] selected via input data
        ow = own_pool.tile([128, HT], mdt, name="ow")
        tmp = stg_pool.tile([128, HT], mdt, name="owt", tag="owt")
        nc.vector.tensor_scalar_mul(tmp, attT[:, 0:HT], sel_sb[:, 2:3])
        nc.vector.tensor_scalar_mul(ow, attT[:, HT:2 * HT], sel_sb[:, 3:4])
        nc.vector.tensor_add(ow, ow, tmp)
        return ow

    def rs_chunk(c, half=None):
        if c == NTC - 1:
            nc.gpsimd.collective_compute(
                "ReduceScatter", mybir.AluOpType.add, replica_groups=PAIRS,
                ins=[agi_3[half][:]], outs=[ago_3[half][:]],
            )
        else:
            nc.gpsimd.collective_compute(
                "ReduceScatter", mybir.AluOpType.add, replica_groups=PAIRS,
                ins=[agi_c[c][:]], outs=[ago_c[c][:]],
            )

    def attention_chunk(c, fillers=None, micro=None):
        nkt = (c + 1) * NSUB
        owns = []
        pending = None
        micro = micro or []
        iters_left = [NDT * nkt]

        def pop_micro():
            if micro:
                npop = -(-len(micro) // max(1, iters_left[0]))
                for _ in range(npop):
                    micro.pop(0)()
            iters_left[0] -= 1

        def finish_stage(pending, pr_done):
            finish_pair(*pending)
            owns.append(stage_pair(c, pr_done, pending[2]))
            if c == NTC - 1 and pr_done == 1:
                rs_chunk(c, half=0)
            if pr_done == NDT - 1:
                rs_chunk(c, half=1 if c == NTC - 1 else None)

        for pr in range(NDT):  # head pair = d-tile
            fn = (fillers or {}).pop(pr, None)
            if fn is not None:
                fn()
            av_ps = [ps_av.tile([HS + 1, TC], f32, name="avp") for _ in range(2)]
            def av_mms(pi, ppt, p0, last):
                # columns below p0 get no contribution from this t_k tile
                # (fully above the diagonal); PSUM accumulation is
                # per-element so the shorter matmul leaves them untouched
                for hh in range(2):
                    nc.tensor.matmul(
                        av_ps[hh][:, p0:TC],
                        mc(vt_c[pi // NSUB][:, pi % NSUB, pr * 2 + hh, :]),
                        mc(ppt[:, hh, p0:TC]),
                        start=(pi == 0), stop=last,
                    )

            avq = []  # stagger AV matmuls 2 units behind exp+mask
            for i in range(nkt):
                m = i - c * NSUB
                # p0: first t_q column this t_k tile can attend to
                p0 = m * TS if m > 0 else 0
                sc2 = ps_sc.tile([TS, 2, TC], f32, name="sc2")
                for hh in range(2):
                    h0 = hh * HS
                    nc.tensor.matmul(
                        sc2[:, hh, p0:TC],
                        mc(kT_c[i // NSUB][h0:h0 + HS, pr,
                                           (i % NSUB) * TS:(i % NSUB + 1) * TS]),
                        mc(qT_c[c][h0:h0 + HS, pr, p0:TC]),
                        start=True, stop=True,
                    )
                pt2 = pt_pool.tile([TS, 2, TC], mdt, name="pt2")
                nc.scalar.activation(
                    out=pt2[:, :, p0:TC], in_=sc2[:, :, p0:TC],
                    func=AF.Exp, scale=SCALE,
                )
                if m >= 0:
                    # diagonal TS block: zero t_k > t_q within it
                    for hh in range(2):
                        nc.vector.tensor_mul(
                            pt2[:, hh, p0:p0 + TS], pt2[:, hh, p0:p0 + TS],
                            mask_sb[:, m, p0:p0 + TS],
                        )
                avq.append((i, pt2, p0))
                if len(avq) > 2:
                    av_mms(*avq.pop(0), last=False)
                if i == 3 and pending is not None:
                    finish_stage(pending, pr - 1)
                    pending = None
                pop_micro()
            while avq:
                av_mms(*avq.pop(0), last=(len(avq) == 0))
            av_sb = avs_pool.tile([HS + 1, 2, TC], mdt, name="av_sb")
            for hh in range(2):
                nc.vector.tensor_copy(av_sb[:, hh, :], av_ps[hh])
            # 1/den on ScalarE as exp(-ln(den)); the two heads' denominator
            # rows are DMA-gathered onto two partitions first so the
            # activation runs on 2 lanes instead of 1
            den2 = dr_pool.tile([2, TC], mdt, name="den2", tag="den")
            for hh in range(2):
                nc.sync.dma_start(
                    out=den2[hh:hh + 1, :], in_=av_sb[HS:HS + 1, hh, :]
                )
            lden = dr_pool.tile([2, TC], f32, name="lden", tag="lden")
            nc.scalar.activation(out=lden, in_=den2, func=AF.Ln)
            dr2 = dr_pool.tile([2, TC], mdt, name="dr2", tag="dr")
            nc.scalar.activation(out=dr2, in_=lden, func=AF.Exp, scale=-1.0)
            attT = attT_pool.tile([128, TC], mdt, name="attT")
            if pending is not None:
                finish_stage(pending, pr - 1)
            pending = (av_sb, dr2, attT)
        while micro:
            micro.pop(0)()
        finish_stage(pending, NDT - 1)
        return owns

    # =====================================================================
    # Phase 3: peer attT + proj + residual + LN2 + FFN on own 256 tokens
    # =====================================================================
    def tail_chunk(c, owns, peer_waits=None):
        """proj for this core's 256-token shard (contraction over own 4 +
        peer 4 d-tiles), then residual + LN2 + FFN + output."""
        # peer/x_own/out go on the gpsimd DMA queue: it carries the
        # collectives, so the peer load orders naturally behind its RS and
        # none of these (which can wait multi-us on data) block the sync
        # queue that carries the attention-critical den/stage transfers
        peer = peer_pool.tile([128, NDT, HT], mdt, name="peer")
        if c == NTC - 1:
            for h in range(2):
                nc.gpsimd.dma_start(
                    out=peer[:, 2 * h:2 * h + 2, :],
                    in_=ago_3[h].rearrange("d p f -> p d f"),
                )
        else:
            nc.gpsimd.dma_start(
                out=peer, in_=ago_c[c].rearrange("d p f -> p d f")
            )
        x2_ts = []
        h2_ts = []
        for s in range(2):
            x2_t = x2_pool.tile([128, E], f32, name="x2_t")
            nc.sync.dma_start(
                out=x2_t, in_=io["x_own"][c, s * TS:(s + 1) * TS, :]
            )
            for n in range(2):
                ps = ps_mm.tile([128, TC], f32, name="ps_pr", tag="mm")
                for dd in range(NDT):
                    nc.tensor.matmul(
                        ps, mc(owns[dd][:, s * TS:(s + 1) * TS]),
                        mc(wp_sb[:, dd, n * TC:(n + 1) * TC]),
                        start=(dd == 0), stop=False,
                    )
                for dd in range(NDT):
                    nc.tensor.matmul(
                        ps, mc(peer[:, dd, s * TS:(s + 1) * TS]),
                        mc(wp_sb[:, NDT + dd, n * TC:(n + 1) * TC]),
                        start=False, stop=(dd == NDT - 1),
                    )
                nc.vector.tensor_add(
                    x2_t[:, n * TC:(n + 1) * TC], x2_t[:, n * TC:(n + 1) * TC],
                    ps,
                )
            x2_ts.append(x2_t)
            h2_t = h_pool.tile([128, E], mdt, name="h2_t", tag="h_t")
            layer_norm(x2_t, h2_t)
            h2_ts.append(h2_t)
        h2T = hT_pool.tile([ET, NET, HT], mdt, name="h2T")
        transpose_cast(h2_ts, ln_sb["ln2g"], ln_sb["ln2b"], h2T, HT)
        f1 = f1_pool.tile([FFN + 1, HT], mdt, name="f1")
        nc.vector.memset(f1, 1.0)  # row FFN stays 1.0 (b2 matmul row)
        ps_f = ps_mm.tile([FFN, HT], f32, name="ps_f", tag="mm")
        for k in range(NET):
            nc.tensor.matmul(
                ps_f, mc(w1_sb[:, k, :]), mc(h2T[:, k, :]),
                start=(k == 0), stop=(k == NET - 1),
            )
        nc.scalar.activation(
            out=f1[0:FFN, :], in_=ps_f, func=AF.Relu, bias=b1_sb, scale=1.0
        )
        for s in range(2):
            o_t = out_pool.tile([128, E], f32, name="o_t")
            for n in range(2):
                ps = ps_mm.tile([128, TC], f32, name="ps_o", tag="mm")
                nc.tensor.matmul(
                    ps, mc(f1[:, s * TS:(s + 1) * TS]),
                    mc(w2_sb[:, n * TC:(n + 1) * TC]),
                    start=True, stop=True,
                )
                nc.vector.tensor_add(
                    o_t[:, n * TC:(n + 1) * TC], ps,
                    x2_ts[s][:, n * TC:(n + 1) * TC],
                )
            nc.sync.dma_start(out=out[c, s * TS:(s + 1) * TS, :], in_=o_t)

    # ---- schedule: minimal QKV-0 prefix (hT + d-tile 0's q/k + V) so
    # attention 0 starts early; the rest of QKV 0 and all of QKV c+1 are
    # micro-interleaved into attention c's iteration stream; chunk c-1's
    # tail (peer+proj+FFN) runs as a filler at pair 2.  A throwaway pair
    # collective right after the weight loads absorbs the first-collective
    # rank-arrival skew + CC-path warmup so RS0 isn't 3x slower. ----
    warmup(12)
    load_late_weights()
    nc.gpsimd.collective_compute(
        "ReduceScatter", mybir.AluOpType.add, replica_groups=PAIRS,
        ins=[al_i[:]], outs=[al_o[:]],
    )
    hT0 = qkv_hT(ln1_from(x0))
    qk_group(0, hT0, 0, wq_sb, qT_c[0])
    qk_group(0, hT0, 0, wk_sb, kT_c[0])
    for s in range(NSUB):
        v_group(0, hT0, s)
    micro = []
    for dd in range(1, NDT):
        micro.append(lambda dd=dd: qk_group(0, hT0, dd, wq_sb, qT_c[0]))
        micro.append(lambda dd=dd: qk_group(0, hT0, dd, wk_sb, kT_c[0]))
    micro += qkv_tasks(1)
    pending_tail = None
    for c in range(NTC):
        fillers = {}
        if pending_tail is not None:
            fillers[2] = pending_tail
        owns = attention_chunk(c, fillers=fillers, micro=micro)
        micro = qkv_tasks(c + 2) if c + 2 < NTC else []
        pending_tail = (lambda cc, oo: lambda: tail_chunk(cc, oo))(c, owns)
    pending_tail()


# =========================================================================
# Host side
# =========================================================================
def _make_masks(np_mdt):
    # masks[p, d, f] = 1 iff t_k <= t_q for the diagonal block at offset d,
    # i.e. f >= 128*d + p  (t_k = 128*i + p, t_q = 512*c + f, i = 4*c + d)
    m = np.zeros((TS, NSUB, TC), dtype=np.float32)
    for d in range(NSUB):
        for p in range(TS):
            m[p, d, d * TS + p:] = 1.0
    return m.astype(np_mdt)


_NC_CACHE = {}
RUN_KWARGS = {}      # test harness may set {"trace": True} for profiling
LAST_RESULT = None   # BassKernelResults of the most recent run


def kernel(x, wq, wk, wv, w_proj, b_proj, w1, b1, w2, b2, ln1_g, ln1_b, ln2_g,
           ln2_b):
    mode = MM_MODE
    np_mdt = _np_mdt(mode)
    if mode not in _NC_CACHE:
        _NC_CACHE[mode] = build(mode)
    nc = _NC_CACHE[mode]

    x = np.asarray(x, np.float32)
    bp = np.asarray(b_proj, np.float32)
    masks = _make_masks(np_mdt)
    identity = np.eye(TS, dtype=np.float32)
    w2e = np.concatenate([np.asarray(w2, np.float32),
                          np.asarray(b2, np.float32)[None, :]], axis=0)
    wp_full = np.asarray(w_proj, np.float32)
    bsel_np = np.zeros((2, TS), np.float32)
    bsel_np[0, 0:HS] = 1.0
    bsel_np[1, HS:TS] = 1.0

    def own_rows(c, g):
        return np.r_[c * TC + g * HT:c * TC + (g + 1) * HT]

    ln1g = np.asarray(ln1_g, np.float32)
    ln1b = np.asarray(ln1_b, np.float32)
    in_maps = []
    for core in range(NCORE):
        b, g = core // 2, core % 2
        sl = slice(g * DSL, (g + 1) * DSL)
        slp = slice((1 - g) * DSL, (2 - g) * DSL)
        x_own = np.stack(
            [x[b, own_rows(c, g), :] for c in range(NTC)]
        ) + bp[None, None, :]
        selmask = np.zeros((128, 4), np.float32)
        selmask[:, 0] = 0.0 if g == 0 else 1.0   # stage: zero own dest slot
        selmask[:, 1] = 0.0 if g == 1 else 1.0
        selmask[:, 2] = 1.0 if g == 0 else 0.0   # own-token half select
        selmask[:, 3] = 1.0 if g == 1 else 0.0
        wp_core = np.concatenate([wp_full[sl, :], wp_full[slp, :]], axis=0)
        in_maps.append({
            "x": x[b].astype(np_mdt),
            "x_own": x_own,
            "selmask": selmask,
            "wq": np.asarray(wq, np.float32)[:, sl].astype(np_mdt),
            "wk": np.asarray(wk, np.float32)[:, sl].astype(np_mdt),
            "wv": np.asarray(wv, np.float32)[:, sl].astype(np_mdt),
            "wp": wp_core.astype(np_mdt),
            "w1": np.asarray(w1, np.float32).astype(np_mdt),
            "w2e": w2e.astype(np_mdt),
            "b1": np.asarray(b1, np.float32)[:, None],
            "ln1g": np.asarray(ln1_g, np.float32)[:, None],
            "ln1b": np.asarray(ln1_b, np.float32)[:, None],
            "ln2g": np.asarray(ln2_g, np.float32)[:, None],
            "ln2b": np.asarray(ln2_b, np.float32)[:, None],
            "masks": masks,
            "ident": identity.astype(np_mdt),
            "bsel": bsel_np.astype(np_mdt),
        })
    global LAST_RESULT
    res = run_bass_kernel_spmd(nc, in_maps, list(range(NCORE)), **RUN_KWARGS)
    LAST_RESULT = res
    outp = np.empty((B, T, E), np.float32)
    for core in range(NCORE):
        b, g = core // 2, core % 2
        o = res.results[core]["out"]
        for c in range(NTC):
            outp[b, own_rows(c, g), :] = o[c]
    return outp


# revision 61
# speedup vs baseline: 1.2166x; 1.0060x over previous
"""Trainium2 Bass kernel: pre-LN transformer block (B=4, T=2048, E=1024, H=16, FFN=100).

Sharding (8 NeuronCores): core 2b+g handles batch b, head-group g (8 of 16
heads, i.e. a 512-wide slice of the QKV output dim / proj input dim).  Both
cores of a pair compute attention for all 2048 tokens of their batch; after
each chunk's attention the pair exchanges normalized attention outputs
(attT) for the tokens the *other* core owns via a zero-masked pair
ReduceScatter (each core's contribution to its own slot is multiplied by a
per-core 0/1 input mask, so the RS-add delivers exactly the peer's attT) —
256KB on the wire per chunk instead of the 1MB proj-partial RS, and it fires
*before* proj, so proj + residual + LN2 + FFN for the core's own 256-token
shard run with no collective behind them.  All rank-dependent choices (which
token half is "mine", the proj weight row order, residual rows) live in
per-core input data, keeping the single SPMD program rank-symmetric.

Schedule: a burst of throwaway warm-up matmuls at t~0 flips the PE HAM clock
gate to 8/8 before real work lands; chunk 0's x subtiles and wq stream first
so LN1+QKV start ~15us in.  All four chunks' LN1+QKV run first (dense PE
work), then attention per chunk; chunk c's tail (peer attT load + proj + FFN)
is emitted inside chunk c+1's attention stream so the PE never idles and
every RS overlaps later attention; the last chunk's RS is split in two so
its tail starts earlier.  x is loaded in bf16 (it only feeds LN1); the f32
residual rows arrive separately as x_own (with b_proj folded in host-side).

Attention: scores are computed transposed, S^T[t_k, t_q] = k^T.T @ q^T, with
q^T/k^T in [head_dim, token] layout (from PE-transposed bf16 LN output).  The
two heads of a d-tile pair occupy partitions 0-63 / 64-127 and run as
concurrent row-group matmuls into one 2-bank PSUM tile, so a single ScalarE
exp (1/sqrt(E) scale folded in) covers both.  Diagonal t_k tiles compute only
the causally live columns (shorter score/AV matmuls + sliced exp) and one
TS-wide mask multiply; AV matmuls trail the exp stream by two tiles.  The
softmax denominator comes from a ones column appended to V; its reciprocal is
exp(-ln(den)) on ScalarE after a tiny SBUF->SBUF DMA gathers both heads'
denominator rows onto two partitions (a [1, N] activation would serialize on
one lane), and is broadcast across the head's 64 partitions with a K=1
ones-matmul into PSUM.  LayerNorm rsqrt is exp(-0.5*ln(var+eps)), keeping
the whole kernel on a single ScalarE table set (natural_log_exp_and_others).
"""

from contextlib import ExitStack

import numpy as np
import ml_dtypes

import concourse.bass as bass
import concourse.mybir as mybir
import concourse.tile as tile
from concourse.bass_utils import run_bass_kernel_spmd
from concourse.vector_clock import ScopedClock


class SplitDrainTC(tile.TileContext):
    """Works around a walrus codegen limit: an SP CTRL instruction may carry
    only one sync wait, so the kernel-tail drain's waits are split onto
    preceding single-wait nops."""

    def _drain_and_barrier(self, tick_clock, wait_clock):
        probe = self.nc.sync.nop(nofuse=True)
        wait_clock.add_sem_waits(
            probe.ins, ScopedClock({None: tick_clock.global_clock})
        )
        si = probe.ins.sync_info
        waits = list(si.on_wait) if si is not None else []
        if len(waits) > 1:
            si.on_wait = [waits[0]]
            for w in waits[1:]:
                n2 = self.nc.sync.nop(nofuse=True)
                n2.ins.sync_info = mybir.SyncInfo(on_wait=[w], on_update=[])
        self.nc.sync.drain()
        self.nc.all_engine_barrier()
        popped = self.nc._tile_sem_poison_stack.pop()
        assert popped is self._sem_poison
        self.nc.clear_and_free_semaphores(list(self.sems.allocated().values()))
        self.nc.all_engine_barrier()

B, T, E, H, HS, FFN = 4, 2048, 1024, 16, 64, 100
EPS = 1e-5
NCORE = 8
TC = 512            # token chunk
NTC = T // TC       # 4
TS = 128            # token subtile
NSUB = TC // TS     # 4
ET = 128            # embed tile
NET = E // ET       # 8
DSL = E // 2        # per-core qkv output slice (8 heads * 64)
NDT = DSL // 128    # 4 d-tiles (2 heads each)
HPC = H // 2        # 8 heads per core
HT = TC // 2        # 256: tokens owned per core per chunk
SCALE = float(E) ** -0.5
PAIRS = [[0, 1], [2, 3], [4, 5], [6, 7]]

MM_MODE = "bf16"    # "bf16" | "f32r" | "f32"
AF = mybir.ActivationFunctionType


def _mdt(mode):
    return mybir.dt.bfloat16 if mode == "bf16" else mybir.dt.float32


def _np_mdt(mode):
    return ml_dtypes.bfloat16 if mode == "bf16" else np.float32


def build(mode=MM_MODE):
    f32 = mybir.dt.float32
    mdt = _mdt(mode)

    def mc(ap):
        """Cast an AP for use as a matmul operand."""
        if mode == "f32r":
            return ap.bitcast(mybir.dt.float32r)
        return ap

    nc = bass.Bass(num_devices=NCORE)

    io = {}

    def param(name, shape, dtype):
        io[name] = nc.declare_dram_parameter(name, shape, dtype, isOutput=False)

    param("x", [T, E], mdt)                  # bf16: only feeds LN1
    param("x_own", [NTC, HT, E], f32)        # own residual rows, + b_proj
    # cols 0-1: RS staging sel (0 for my own dest slot); cols 2-3: own-token
    # half select (1 for my half)
    param("selmask", [128, 4], f32)
    param("wq", [E, DSL], mdt)
    param("wk", [E, DSL], mdt)
    param("wv", [E, DSL], mdt)
    param("wp", [E, E], mdt)                 # rows reordered: [own 512; peer 512]
    param("w1", [E, FFN], mdt)
    param("w2e", [FFN + 1, E], mdt)    # w2 with b2 as the extra last row
    param("b1", [FFN, 1], f32)
    param("ln1g", [E, 1], f32)
    param("ln1b", [E, 1], f32)
    param("ln2g", [E, 1], f32)
    param("ln2b", [E, 1], f32)
    param("masks", [TS, NSUB, TC], mdt)
    param("ident", [TS, TS], mdt)
    param("bsel", [2, TS], mdt)
    io["out"] = nc.declare_dram_parameter(
        "out", [NTC, HT, E], f32, isOutput=True
    )

    with SplitDrainTC(nc) as tc:
        with ExitStack() as ctx:
            _build_tile(ctx, tc, nc, mode, mdt, f32, mc, io)
    _split_waits(nc)
    return nc


def _split_waits(nc, maxw=1):
    """walrus codegen accepts a limited number of sync waits per instruction;
    move the excess onto same-engine NoOps inserted just before."""
    import bass_rust
    n = 0
    for f in nc.m.functions:
        for b in f.blocks:
            new = []
            for inst in b.instructions:
                si = inst.sync_info
                # fixed-length ISA instructions can't carry waits at all
                cap = 0 if isinstance(inst, bass_rust.InstISA) else maxw
                if si is not None and len(si.on_wait) > cap:
                    waits = list(si.on_wait)
                    keep = waits[-cap:] if cap else []
                    excess = waits[:-cap] if cap else waits
                    for w in excess:
                        nop = mybir.InstNoOp(
                            name=f"{inst.name}-wsplit{n}", engine=inst.engine
                        )
                        nop.bass_nofuse = True
                        n += 1
                        nop.sync_info = mybir.SyncInfo(
                            on_wait=[w], on_update=[]
                        )
                        new.append(nop)
                    si.on_wait = keep
                new.append(inst)
            if n:
                b.instructions = new


def _build_tile(ctx, tc, nc, mode, mdt, f32, mc, io):
    x, out = io["x"], io["out"]

    def pool(name, bufs, space="SBUF"):
        return ctx.enter_context(tc.tile_pool(name=name, bufs=bufs, space=space))

    # ---- internal DRAM: per-chunk attT-exchange RS buffers.  agi[j] holds
    # this core's attT columns for the tokens rank j owns (own-dest slot
    # zero-masked); the pair RS-add delivers the peer's attT for my tokens.
    dram = pool("dram", 1, space="DRAM")
    al_i = dram.tile([2, 128], mybir.dt.float32, name="al_i")
    al_o = dram.tile([1, 128], mybir.dt.float32, name="al_o")
    agi_c = [dram.tile([2, NDT, TS, HT], mdt, name=f"agi{c}") for c in range(3)]
    ago_c = [dram.tile([NDT, TS, HT], mdt, name=f"ago{c}") for c in range(3)]
    # last chunk: two half-exchanges (d-tiles 0-1 / 2-3) so its tail starts
    # as soon as the first half's attention pairs finish
    agi_3 = [dram.tile([2, 2, TS, HT], mdt, name=f"agi3{h}") for h in range(2)]
    ago_3 = [dram.tile([2, TS, HT], mdt, name=f"ago3{h}") for h in range(2)]

    # ---- persistent SBUF: weights & constants.  Emission order sets the
    # tile scheduler's priority: chunk 0's x subtiles + the small consts +
    # wq go first so LN1+QKV start early. ----
    wpool = pool("weights", 1)
    xt_pool = pool("xt", 4)        # [128, E] bf16: x rows for LN1

    def prefetch_x(c, spread=False):
        # at startup all queues are empty: spread chunk 0's subtiles over
        # four engine queues so the transfers run in parallel
        qs = [nc.sync, nc.scalar, nc.gpsimd, nc.sync] if spread \
            else [nc.sync] * NSUB
        x_ts = []
        for s in range(NSUB):
            r0 = c * TC + s * TS
            x_t = xt_pool.tile([128, E], mdt, name="x_t")
            qs[s].dma_start(out=x_t, in_=x[r0:r0 + TS, :])
            x_ts.append(x_t)
        return x_ts

    x0 = prefetch_x(0, spread=True)
    warm_src = wpool.tile([128, 128], mdt, name="warm_src")
    nc.vector.memset(warm_src, 0.25)
    warm_mv = wpool.tile([128, TC], mdt, name="warm_mv")
    nc.vector.memset(warm_mv, 0.25)
    ln_sb = {}
    for nm in ("ln1g", "ln1b"):
        t = wpool.tile([ET, NET], f32, name=nm + "_sb")
        nc.scalar.dma_start(
            out=t, in_=io[nm].rearrange("(k p) o -> p (k o)", p=ET)
        )
        ln_sb[nm] = t[:, :]
    sel_sb = wpool.tile([128, 4], f32, name="sel_sb")
    nc.scalar.dma_start(out=sel_sb, in_=io["selmask"][:])
    id_sb = wpool.tile([TS, TS], mdt, name="id_sb")
    nc.scalar.dma_start(out=id_sb, in_=io["ident"][:])
    # block "selector" for the denominator broadcast: one K=2 matmul maps
    # dr2 [2, TC] onto [128, TC] with head h's reciprocal on partitions
    # h*64..h*64+63 (host-provided constant)
    bsel = wpool.tile([2, 128], mdt, name="bsel")
    nc.scalar.dma_start(out=bsel, in_=io["bsel"][:])
    wq_sb = wpool.tile([ET, NET, DSL], mdt, name="wq_sb")
    wk_sb = wpool.tile([ET, NET, DSL], mdt, name="wk_sb")
    wv_sb = wpool.tile([ET, NET, DSL], mdt, name="wv_sb")
    nc.gpsimd.dma_start(out=wq_sb, in_=io["wq"].rearrange("(k p) d -> p k d", p=ET))
    nc.gpsimd.dma_start(out=wk_sb, in_=io["wk"].rearrange("(k p) d -> p k d", p=ET))
    nc.gpsimd.dma_start(out=wv_sb, in_=io["wv"].rearrange("(k p) d -> p k d", p=ET))
    eps_sb = wpool.tile([128, 1], f32, name="eps_sb")
    nc.vector.memset(eps_sb, EPS)
    mask_sb = wpool.tile([TS, NSUB, TC], mdt, name="mask_sb")
    wp_sb = wpool.tile([128, 2 * NDT, E], mdt, name="wp_sb")
    w1_sb = wpool.tile([ET, NET, FFN], mdt, name="w1_sb")
    w2_sb = wpool.tile([FFN + 1, E], mdt, name="w2_sb")
    b1_sb = wpool.tile([FFN, 1], f32, name="b1_sb")

    def load_late_weights():
        nc.gpsimd.dma_start(out=mask_sb, in_=io["masks"][:])
        nc.gpsimd.dma_start(
            out=wp_sb, in_=io["wp"].rearrange("(k p) d -> p k d", p=128)
        )
        nc.gpsimd.dma_start(
            out=w1_sb, in_=io["w1"].rearrange("(k p) d -> p k d", p=ET)
        )
        nc.gpsimd.dma_start(out=w2_sb, in_=io["w2e"][:])
        nc.gpsimd.dma_start(out=b1_sb, in_=io["b1"][:])
        for nm in ("ln2g", "ln2b"):
            t = wpool.tile([ET, NET], f32, name=nm + "_sb")
            nc.gpsimd.dma_start(
                out=t, in_=io[nm].rearrange("(k p) o -> p (k o)", p=ET)
            )
            ln_sb[nm] = t[:, :]

    # ---- persistent SBUF: per-chunk K^T, V(+ones), Q^T ----
    kv = pool("kv", 1)
    kT_c = [kv.tile([128, NDT, TC], mdt, name=f"kT{c}") for c in range(NTC)]
    vt_c = [kv.tile([128, NSUB, HPC, HS + 1], mdt, name=f"vt{c}")
            for c in range(NTC)]
    qT_c = [kv.tile([128, NDT, TC], mdt, name=f"qT{c}") for c in range(NTC)]

    # ---- working pools ----
    h_pool = pool("h", 6)          # [128, E] bf16: LN output rows
    mv_pool = pool("mv", 3)
    hT_pool = pool("hT", 2)        # [128, NET, TC] bf16
    pt_pool = pool("pt", 5)        # [128, 2, TC] bf16 softmax tiles
    avs_pool = pool("avs", 3)      # [HS+1, 2, TC] bf16
    dr_pool = pool("dr", 2)        # [2, TC] denominators / reciprocals
    attT_pool = pool("attT", 5)    # [128, TC] bf16
    own_pool = pool("own", 6)      # [128, HT] bf16: own-token attT columns
    stg_pool = pool("stg", 3)      # [128, 2, HT] bf16: RS staging
    peer_pool = pool("peer", 2)    # [128, NDT, HT] bf16: peer attT
    x2_pool = pool("x2", 2)        # [128, E] f32: phase-3 residual rows
    f1_pool = pool("f1", 2)
    out_pool = pool("outp", 2)
    ps_mm = pool("ps_mm", 2, space="PSUM")   # [128, 512] (1 bank each)
    ps_sc = pool("ps_sc", 2, space="PSUM")   # [128, 2, 512] (2 banks each)
    ps_av = pool("ps_av", 2, space="PSUM")   # [HS+1, 512] (1 bank each)

    # ---- HAM warm-up: ~9us of back-to-back throwaway matmuls (one PSUM
    # accumulation group -> no inter-matmul semaphores) so the PE clock gate
    # is at 8/8 and stays there until the first real QKV work arrives ----
    def warmup(n=22):
        psw = ps_mm.tile([128, TC], f32, name="psw", tag="mm")
        for i in range(n):
            nc.tensor.matmul(psw, mc(warm_src), mc(warm_mv),
                             start=(i == 0), stop=(i == n - 1))

    def layer_norm(x_t, out_t):
        """out_t (bf16) = (x - mean) * rsqrt(var + eps).
        rsqrt is exp(-0.5*ln(var+eps)) to stay in one ScalarE table set."""
        stats = mv_pool.tile([128, 2, nc.vector.BN_STATS_DIM], f32, name="stats")
        xg = x_t.rearrange("p (s q) -> p s q", s=2)
        for s in range(2):
            nc.vector.bn_stats(out=stats[:, s, :], in_=xg[:, s, :])
        mv = mv_pool.tile([128, 2], f32, name="mv")
        nc.vector.bn_aggr(out=mv, in_=stats)
        rstd = mv_pool.tile([128, 1], f32, name="rstd")
        nc.scalar.activation(
            out=rstd, in_=mv[:, 1:2], func=AF.Ln, bias=eps_sb, scale=1.0
        )
        nc.scalar.activation(out=rstd, in_=rstd, func=AF.Exp, scale=-0.5)
        nc.vector.tensor_scalar(
            out=out_t, in0=x_t, scalar1=mv[:, 0:1], scalar2=rstd,
            op0=mybir.AluOpType.subtract, op1=mybir.AluOpType.mult,
        )
        return out_t

    def transpose_cast(h_ts, g_sb, b_sb, hT, width):
        """Transpose len(h_ts) subtiles of h [128, E] into hT[:, k, :]
        (bf16), batching all of them into one PSUM tile per e-tile so the
        layernorm scale/bias fold costs one DVE op per [128, width].
        The transpose is a *regular* matmul against the identity (out =
        h.T @ I): transpose-mode matmuls cost ~275ns each (SBUF access
        latency, and they don't count as HAM activity); normal-mode runs
        ~107ns and keeps the clock gate warm."""
        nsub = len(h_ts)
        for k in range(NET):
            tp = ps_mm.tile([TS, nsub * TS], f32, name="tp", tag="mm")
            for s in range(nsub):
                nc.tensor.matmul(
                    tp[:, s * TS:(s + 1) * TS],
                    mc(h_ts[s][:, k * ET:(k + 1) * ET]), mc(id_sb),
                    start=True, stop=True,
                )
            nc.vector.tensor_scalar(
                out=hT[:, k, 0:width], in0=tp,
                scalar1=g_sb[:, k:k + 1], scalar2=b_sb[:, k:k + 1],
                op0=mybir.AluOpType.mult, op1=mybir.AluOpType.add,
            )

    # =====================================================================
    # Phase 1: LN1 + transpose + QKV per chunk
    # =====================================================================
    def layer_norm_sc(x_t, out_t):
        """LayerNorm with the two reduction passes on ScalarE (sum and
        sum-of-squares via activation accum_out) instead of DVE bn_stats;
        used for chunk 0 where ScalarE is otherwise idle, halving the
        serial LN1 latency by splitting subtiles across both engines."""
        sums = mv_pool.tile([128, 2], f32, name="sums")
        scr = pt_pool.tile([TS, 2, TC], mdt, name="pt2")
        nc.scalar.activation(
            out=out_t, in_=x_t, func=AF.Copy, accum_out=sums[:, 0:1]
        )
        nc.scalar.activation(
            out=scr.rearrange("p a f -> p (a f)"), in_=x_t, func=AF.Square,
            accum_out=sums[:, 1:2],
        )
        mv2 = mv_pool.tile([128, 2], f32, name="mv")
        nc.vector.tensor_scalar_mul(mv2, sums, 1.0 / E)
        var = mv_pool.tile([128, 1], f32, name="var")
        nc.vector.tensor_mul(var, mv2[:, 0:1], mv2[:, 0:1])
        nc.vector.tensor_sub(var, mv2[:, 1:2], var)
        rstd = mv_pool.tile([128, 1], f32, name="rstd")
        nc.scalar.activation(
            out=rstd, in_=var, func=AF.Ln, bias=eps_sb, scale=1.0
        )
        nc.scalar.activation(out=rstd, in_=rstd, func=AF.Exp, scale=-0.5)
        nc.vector.tensor_scalar(
            out=out_t, in0=x_t, scalar1=mv2[:, 0:1], scalar2=rstd,
            op0=mybir.AluOpType.subtract, op1=mybir.AluOpType.mult,
        )
        return out_t

    def ln1_from(x_ts, split=False):
        h_ts = []
        for s, x_t in enumerate(x_ts):
            h_t = h_pool.tile([128, E], mdt, name="h_t")
            if split and s % 2 == 1:
                h_ts.append(layer_norm_sc(x_t, h_t))
            else:
                h_ts.append(layer_norm(x_t, h_t))
        return h_ts

    def qkv_hT(h_ts):
        hT = hT_pool.tile([ET, NET, TC], mdt, name="hT")
        transpose_cast(h_ts, ln_sb["ln1g"], ln_sb["ln1b"], hT, TC)
        return hT

    def qk_group(c, hT, dd, w_sb, dst):
        ps = ps_mm.tile([128, TC], f32, name="ps_qk", tag="mm")
        for k in range(NET):
            nc.tensor.matmul(
                ps, mc(w_sb[:, k, dd * 128:(dd + 1) * 128]),
                mc(hT[:, k, :]),
                start=(k == 0), stop=(k == NET - 1),
            )
        nc.vector.tensor_copy(dst[:, dd, :], ps)

    def v_group(c, hT, s):
        ps = ps_mm.tile([128, DSL], f32, name="ps_v", tag="mm")
        for k in range(NET):
            nc.tensor.matmul(
                ps, mc(hT[:, k, s * TS:(s + 1) * TS]), mc(wv_sb[:, k, :]),
                start=(k == 0), stop=(k == NET - 1),
            )
        nc.vector.tensor_copy(
            vt_c[c][:, s, :, 0:HS],
            ps.rearrange("p (h d) -> p h d", h=HPC),
        )
        nc.vector.memset(vt_c[c][:, s, :, HS:HS + 1], 1.0)

    def qkv_mms(c, hT):
        for dd in range(NDT):
            qk_group(c, hT, dd, wq_sb, qT_c[c])
            qk_group(c, hT, dd, wk_sb, kT_c[c])
        for s in range(NSUB):
            v_group(c, hT, s)

    def qkv_tasks(c):
        """Chunk c's LN1+transpose+QKV as a list of small closures, popped
        one-or-two per attention iteration of chunk c-1 so the PE stream
        stays dense (and the HAM clock warm) through the ScalarE-bound
        attention phase."""
        x_ts = prefetch_x(c)
        h_ts = []
        hT = hT_pool.tile([ET, NET, TC], mdt, name="hT")
        tasks = []

        def ln_task(s):
            def f():
                h_t = h_pool.tile([128, E], mdt, name="h_t")
                h_ts.append(layer_norm(x_ts[s], h_t))
            return f

        def tp_task(k):
            def f():
                tp = ps_mm.tile([TS, NSUB * TS], f32, name="tp", tag="mm")
                for s in range(NSUB):
                    nc.tensor.matmul(
                        tp[:, s * TS:(s + 1) * TS],
                        mc(h_ts[s][:, k * ET:(k + 1) * ET]), mc(id_sb),
                        start=True, stop=True,
                    )
                nc.vector.tensor_scalar(
                    out=hT[:, k, :], in0=tp,
                    scalar1=ln_sb["ln1g"][:, k:k + 1],
                    scalar2=ln_sb["ln1b"][:, k:k + 1],
                    op0=mybir.AluOpType.mult, op1=mybir.AluOpType.add,
                )
            return f

        for s in range(NSUB):
            tasks.append(ln_task(s))
        for k in range(NET):
            tasks.append(tp_task(k))
        for dd in range(NDT):
            tasks.append(lambda dd=dd: qk_group(c, hT, dd, wq_sb, qT_c[c]))
            tasks.append(lambda dd=dd: qk_group(c, hT, dd, wk_sb, kT_c[c]))
        for s in range(NSUB):
            tasks.append(lambda s=s: v_group(c, hT, s))
        return tasks

    # =====================================================================
    # Phase 2: attention + attT normalize + pair exchange
    # =====================================================================
    def finish_pair(av_sb, dr2, attT):
        """Broadcast the reciprocal denominators across each head's 64
        partitions with a K=2 selector-matmul and normalize into attT."""
        rb = ps_mm.tile([128, TC], f32, name="rb", tag="mm")
        nc.tensor.matmul(rb, mc(bsel), mc(dr2), start=True, stop=True)
        for hh in range(2):
            nc.vector.tensor_mul(
                attT[hh * HS:(hh + 1) * HS, :], av_sb[0:HS, hh, :],
                rb[hh * HS:(hh + 1) * HS, :],
            )

    def stage_pair(c, pr, attT):
        """Write attT's two token-half column blocks to the RS staging DRAM
        (own-dest slot zeroed via selmask) and produce the own-token columns
        for the local proj."""
        stg = stg_pool.tile([128, 2, HT], mdt, name="stg")
        for j in range(2):
            nc.vector.tensor_scalar_mul(
                stg[:, j, :], attT[:, j * HT:(j + 1) * HT],
                sel_sb[:, j:j + 1],
            )
        if c == NTC - 1:
            dst = agi_3[pr // 2][:, pr % 2, :, :]
        else:
            dst = agi_c[c][:, pr, :, :]
        nc.sync.dma_start(out=dst.rearrange("j p f -> p j f"), in_=stg)
        # own-token columns: attT[:, g*256:(g+1)*256] selected via input data
        ow = own_pool.tile([128, HT], mdt, name="ow")
        tmp = stg_pool.tile([128, HT], mdt, name="owt", tag="owt")
        nc.vector.tensor_scalar_mul(tmp, attT[:, 0:HT], sel_sb[:, 2:3])
        nc.vector.tensor_scalar_mul(ow, attT[:, HT:2 * HT], sel_sb[:, 3:4])
        nc.vector.tensor_add(ow, ow, tmp)
        return ow

    def rs_chunk(c, half=None):
        if c == NTC - 1:
            nc.gpsimd.collective_compute(
                "ReduceScatter", mybir.AluOpType.add, replica_groups=PAIRS,
                ins=[agi_3[half][:]], outs=[ago_3[half][:]],
            )
        else:
            nc.gpsimd.collective_compute(
                "ReduceScatter", mybir.AluOpType.add, replica_groups=PAIRS,
                ins=[agi_c[c][:]], outs=[ago_c[c][:]],
            )

    def attention_chunk(c, fillers=None, micro=None):
        nkt = (c + 1) * NSUB
        owns = []
        pending = None
        micro = micro or []
        iters_left = [NDT * nkt]

        def pop_micro():
            if micro:
                npop = -(-len(micro) // max(1, iters_left[0]))
                for _ in range(npop):
                    micro.pop(0)()
            else:
                # no QKV work left (late chunks): keep the PE HAM clock at
                # 8/8 through the ScalarE-bound stretch with throwaway
                # weight loads (no PSUM, no dependencies)
                nc.tensor.ldweights(warm_src[:])
                nc.tensor.ldweights(warm_src[:])
            iters_left[0] -= 1

        def finish_stage(pending, pr_done):
            finish_pair(*pending)
            owns.append(stage_pair(c, pr_done, pending[2]))
            if c == NTC - 1 and pr_done == 1:
                rs_chunk(c, half=0)
            if pr_done == NDT - 1:
                rs_chunk(c, half=1 if c == NTC - 1 else None)

        for pr in range(NDT):  # head pair = d-tile
            fn = (fillers or {}).pop(pr, None)
            if fn is not None:
                fn()
            av_ps = [ps_av.tile([HS + 1, TC], f32, name="avp") for _ in range(2)]
            def av_mms(pi, ppt, p0, last):
                # columns below p0 get no contribution from this t_k tile
                # (fully above the diagonal); PSUM accumulation is
                # per-element so the shorter matmul leaves them untouched
                for hh in range(2):
                    nc.tensor.matmul(
                        av_ps[hh][:, p0:TC],
                        mc(vt_c[pi // NSUB][:, pi % NSUB, pr * 2 + hh, :]),
                        mc(ppt[:, hh, p0:TC]),
                        start=(pi == 0), stop=last,
                    )

            avq = []  # stagger AV matmuls 2 units behind exp+mask
            for i in range(nkt):
                m = i - c * NSUB
                # p0: first t_q column this t_k tile can attend to
                p0 = m * TS if m > 0 else 0
                sc2 = ps_sc.tile([TS, 2, TC], f32, name="sc2")
                for hh in range(2):
                    h0 = hh * HS
                    nc.tensor.matmul(
                        sc2[:, hh, p0:TC],
                        mc(kT_c[i // NSUB][h0:h0 + HS, pr,
                                           (i % NSUB) * TS:(i % NSUB + 1) * TS]),
                        mc(qT_c[c][h0:h0 + HS, pr, p0:TC]),
                        start=True, stop=True,
                    )
                pt2 = pt_pool.tile([TS, 2, TC], mdt, name="pt2")
                nc.scalar.activation(
                    out=pt2[:, :, p0:TC], in_=sc2[:, :, p0:TC],
                    func=AF.Exp, scale=SCALE,
                )
                if m >= 0:
                    # diagonal TS block: zero t_k > t_q within it
                    for hh in range(2):
                        nc.vector.tensor_mul(
                            pt2[:, hh, p0:p0 + TS], pt2[:, hh, p0:p0 + TS],
                            mask_sb[:, m, p0:p0 + TS],
                        )
                avq.append((i, pt2, p0))
                if len(avq) > 2:
                    av_mms(*avq.pop(0), last=False)
                if i == 3 and pending is not None:
                    finish_stage(pending, pr - 1)
                    pending = None
                pop_micro()
            while avq:
                av_mms(*avq.pop(0), last=(len(avq) == 0))
            av_sb = avs_pool.tile([HS + 1, 2, TC], mdt, name="av_sb")
            for hh in range(2):
                nc.vector.tensor_copy(av_sb[:, hh, :], av_ps[hh])
            # 1/den on ScalarE as exp(-ln(den)); the two heads' denominator
            # rows are DMA-gathered onto two partitions first so the
            # activation runs on 2 lanes instead of 1
            den2 = dr_pool.tile([2, TC], mdt, name="den2", tag="den")
            for hh in range(2):
                nc.sync.dma_start(
                    out=den2[hh:hh + 1, :], in_=av_sb[HS:HS + 1, hh, :]
                )
            lden = dr_pool.tile([2, TC], f32, name="lden", tag="lden")
            nc.scalar.activation(out=lden, in_=den2, func=AF.Ln)
            dr2 = dr_pool.tile([2, TC], mdt, name="dr2", tag="dr")
            nc.scalar.activation(out=dr2, in_=lden, func=AF.Exp, scale=-1.0)
            attT = attT_pool.tile([128, TC], mdt, name="attT")
            if pending is not None:
                finish_stage(pending, pr - 1)
            pending = (av_sb, dr2, attT)
        while micro:
            micro.pop(0)()
        finish_stage(pending, NDT - 1)
        return owns

    # =====================================================================
    # Phase 3: peer attT + proj + residual + LN2 + FFN on own 256 tokens
    # =====================================================================
    def tail_chunk(c, owns, peer_waits=None):
        """proj for this core's 256-token shard (contraction over own 4 +
        peer 4 d-tiles), then residual + LN2 + FFN + output."""
        # peer/x_own/out go on the gpsimd DMA queue: it carries the
        # collectives, so the peer load orders naturally behind its RS and
        # none of these (which can wait multi-us on data) block the sync
        # queue that carries the attention-critical den/stage transfers
        peer = peer_pool.tile([128, NDT, HT], mdt, name="peer")
        if c == NTC - 1:
            for h in range(2):
                nc.gpsimd.dma_start(
                    out=peer[:, 2 * h:2 * h + 2, :],
                    in_=ago_3[h].rearrange("d p f -> p d f"),
                )
        else:
            nc.gpsimd.dma_start(
                out=peer, in_=ago_c[c].rearrange("d p f -> p d f")
            )
        x2_ts = []
        h2_ts = []
        for s in range(2):
            x2_t = x2_pool.tile([128, E], f32, name="x2_t")
            nc.sync.dma_start(
                out=x2_t, in_=io["x_own"][c, s * TS:(s + 1) * TS, :]
            )
            for n in range(2):
                ps = ps_mm.tile([128, TC], f32, name="ps_pr", tag="mm")
                for dd in range(NDT):
                    nc.tensor.matmul(
                        ps, mc(owns[dd][:, s * TS:(s + 1) * TS]),
                        mc(wp_sb[:, dd, n * TC:(n + 1) * TC]),
                        start=(dd == 0), stop=False,
                    )
                for dd in range(NDT):
                    nc.tensor.matmul(
                        ps, mc(peer[:, dd, s * TS:(s + 1) * TS]),
                        mc(wp_sb[:, NDT + dd, n * TC:(n + 1) * TC]),
                        start=False, stop=(dd == NDT - 1),
                    )
                nc.vector.tensor_add(
                    x2_t[:, n * TC:(n + 1) * TC], x2_t[:, n * TC:(n + 1) * TC],
                    ps,
                )
            x2_ts.append(x2_t)
            h2_t = h_pool.tile([128, E], mdt, name="h2_t", tag="h_t")
            layer_norm(x2_t, h2_t)
            h2_ts.append(h2_t)
        h2T = hT_pool.tile([ET, NET, HT], mdt, name="h2T")
        transpose_cast(h2_ts, ln_sb["ln2g"], ln_sb["ln2b"], h2T, HT)
        f1 = f1_pool.tile([FFN + 1, HT], mdt, name="f1")
        nc.vector.memset(f1, 1.0)  # row FFN stays 1.0 (b2 matmul row)
        ps_f = ps_mm.tile([FFN, HT], f32, name="ps_f", tag="mm")
        for k in range(NET):
            nc.tensor.matmul(
                ps_f, mc(w1_sb[:, k, :]), mc(h2T[:, k, :]),
                start=(k == 0), stop=(k == NET - 1),
            )
        nc.scalar.activation(
            out=f1[0:FFN, :], in_=ps_f, func=AF.Relu, bias=b1_sb, scale=1.0
        )
        for s in range(2):
            o_t = out_pool.tile([128, E], f32, name="o_t")
            for n in range(2):
                ps = ps_mm.tile([128, TC], f32, name="ps_o", tag="mm")
                nc.tensor.matmul(
                    ps, mc(f1[:, s * TS:(s + 1) * TS]),
                    mc(w2_sb[:, n * TC:(n + 1) * TC]),
                    start=True, stop=True,
                )
                nc.vector.tensor_add(
                    o_t[:, n * TC:(n + 1) * TC], ps,
                    x2_ts[s][:, n * TC:(n + 1) * TC],
                )
            nc.sync.dma_start(out=out[c, s * TS:(s + 1) * TS, :], in_=o_t)

    # ---- schedule: minimal QKV-0 prefix (hT + d-tile 0's q/k + V) so
    # attention 0 starts early; the rest of QKV 0 and all of QKV c+1 are
    # micro-interleaved into attention c's iteration stream; chunk c-1's
    # tail (peer+proj+FFN) runs as a filler at pair 2.  A throwaway pair
    # collective right after the weight loads absorbs the first-collective
    # rank-arrival skew + CC-path warmup so RS0 isn't 3x slower. ----
    warmup(12)
    load_late_weights()
    nc.gpsimd.collective_compute(
        "ReduceScatter", mybir.AluOpType.add, replica_groups=PAIRS,
        ins=[al_i[:]], outs=[al_o[:]],
    )
    hT0 = qkv_hT(ln1_from(x0, split=True))
    qk_group(0, hT0, 0, wq_sb, qT_c[0])
    qk_group(0, hT0, 0, wk_sb, kT_c[0])
    v_group(0, hT0, 0)
    micro = []
    for dd in range(1, NDT):
        micro.append(lambda dd=dd: qk_group(0, hT0, dd, wq_sb, qT_c[0]))
        micro.append(lambda dd=dd: qk_group(0, hT0, dd, wk_sb, kT_c[0]))
    for s in range(1, NSUB):
        micro.append(lambda s=s: v_group(0, hT0, s))
    micro += qkv_tasks(1)
    pending_tail = None
    for c in range(NTC):
        fillers = {}
        if pending_tail is not None:
            fillers[2] = pending_tail
        owns = attention_chunk(c, fillers=fillers, micro=micro)
        micro = qkv_tasks(c + 2) if c + 2 < NTC else []
        pending_tail = (lambda cc, oo: lambda: tail_chunk(cc, oo))(c, owns)
    pending_tail()


# =========================================================================
# Host side
# =========================================================================
def _make_masks(np_mdt):
    # masks[p, d, f] = 1 iff t_k <= t_q for the diagonal block at offset d,
    # i.e. f >= 128*d + p  (t_k = 128*i + p, t_q = 512*c + f, i = 4*c + d)
    m = np.zeros((TS, NSUB, TC), dtype=np.float32)
    for d in range(NSUB):
        for p in range(TS):
            m[p, d, d * TS + p:] = 1.0
    return m.astype(np_mdt)


_NC_CACHE = {}
RUN_KWARGS = {}      # test harness may set {"trace": True} for profiling
LAST_RESULT = None   # BassKernelResults of the most recent run


def kernel(x, wq, wk, wv, w_proj, b_proj, w1, b1, w2, b2, ln1_g, ln1_b, ln2_g,
           ln2_b):
    mode = MM_MODE
    np_mdt = _np_mdt(mode)
    if mode not in _NC_CACHE:
        _NC_CACHE[mode] = build(mode)
    nc = _NC_CACHE[mode]

    x = np.asarray(x, np.float32)
    bp = np.asarray(b_proj, np.float32)
    masks = _make_masks(np_mdt)
    identity = np.eye(TS, dtype=np.float32)
    w2e = np.concatenate([np.asarray(w2, np.float32),
                          np.asarray(b2, np.float32)[None, :]], axis=0)
    wp_full = np.asarray(w_proj, np.float32)
    bsel_np = np.zeros((2, TS), np.float32)
    bsel_np[0, 0:HS] = 1.0
    bsel_np[1, HS:TS] = 1.0

    def own_rows(c, g):
        return np.r_[c * TC + g * HT:c * TC + (g + 1) * HT]

    ln1g = np.asarray(ln1_g, np.float32)
    ln1b = np.asarray(ln1_b, np.float32)
    in_maps = []
    for core in range(NCORE):
        b, g = core // 2, core % 2
        sl = slice(g * DSL, (g + 1) * DSL)
        slp = slice((1 - g) * DSL, (2 - g) * DSL)
        x_own = np.stack(
            [x[b, own_rows(c, g), :] for c in range(NTC)]
        ) + bp[None, None, :]
        selmask = np.zeros((128, 4), np.float32)
        selmask[:, 0] = 0.0 if g == 0 else 1.0   # stage: zero own dest slot
        selmask[:, 1] = 0.0 if g == 1 else 1.0
        selmask[:, 2] = 1.0 if g == 0 else 0.0   # own-token half select
        selmask[:, 3] = 1.0 if g == 1 else 0.0
        wp_core = np.concatenate([wp_full[sl, :], wp_full[slp, :]], axis=0)
        in_maps.append({
            "x": x[b].astype(np_mdt),
            "x_own": x_own,
            "selmask": selmask,
            "wq": np.asarray(wq, np.float32)[:, sl].astype(np_mdt),
            "wk": np.asarray(wk, np.float32)[:, sl].astype(np_mdt),
            "wv": np.asarray(wv, np.float32)[:, sl].astype(np_mdt),
            "wp": wp_core.astype(np_mdt),
            "w1": np.asarray(w1, np.float32).astype(np_mdt),
            "w2e": w2e.astype(np_mdt),
            "b1": np.asarray(b1, np.float32)[:, None],
            "ln1g": np.asarray(ln1_g, np.float32)[:, None],
            "ln1b": np.asarray(ln1_b, np.float32)[:, None],
            "ln2g": np.asarray(ln2_g, np.float32)[:, None],
            "ln2b": np.asarray(ln2_b, np.float32)[:, None],
            "masks": masks,
            "ident": identity.astype(np_mdt),
            "bsel": bsel_np.astype(np_mdt),
        })
    global LAST_RESULT
    res = run_bass_kernel_spmd(nc, in_maps, list(range(NCORE)), **RUN_KWARGS)
    LAST_RESULT = res
    outp = np.empty((B, T, E), np.float32)
    for core in range(NCORE):
        b, g = core // 2, core % 2
        o = res.results[core]["out"]
        for c in range(NTC):
            outp[b, own_rows(c, g), :] = o[c]
    return outp
